# revision 17
# baseline (speedup 1.0000x reference)
"""ALSHConvNet forward on 8 TRN2 NeuronCores — fast path (a1 <= 4).

Key ideas vs baseline:
- L1 as banded-K matmul: K=108 (3ic x 36iy), M=112 (y-parity split at
  partition 64), 80 streaming passes total (was 640 instructions).
- m1 (host-known) compacts conv2's input channels: K=8*a1<=32 per
  y-window, 4 windows row-packed via tile_position -> 4x fewer passes.
- Q2/s2 kept UNMASKED; m2 folded into conv3 weights (W3M) so the L2->L3
  data path never waits on the s1 AllReduce result for activations.
- x-outer staging layouts; pooling via cross-partition DVE ops (no DMA
  round-trips except L3 parity + staging).
"""
import sys
sys.path.insert(0, '/opt/trn_rl_repo')

import numpy as np
import ml_dtypes

import concourse.bacc as bacc
import concourse.mybir as mybir
import concourse.tile as tile
from concourse.bass_utils import run_bass_kernel_spmd

N_CORES = 8
B_LOC = 256
fp32 = mybir.dt.float32
bf16 = mybir.dt.bfloat16
AL = mybir.AluOpType
AF = mybir.ActivationFunctionType
AX = mybir.AxisListType
BF = ml_dtypes.bfloat16

U = 0.9999
M_APPEND = 3
TABLE = 25


# ---------------------------------------------------------------- host math
def _filter_codes(W, A):
    out_ch = W.shape[0]
    Wf = np.asarray(W, np.float32).reshape(out_ch, -1)
    norms = np.sqrt((Wf * Wf).sum(1))
    Wp = Wf * (U / norms.max())
    n2 = (Wp * Wp).sum(1)
    terms = np.stack([0.5 - n2 ** (2 ** i) for i in range(M_APPEND)], 1)
    P = np.concatenate([Wp, terms], 1).astype(np.float32)
    zW = np.einsum('lkd,nd->lkn', np.asarray(A, np.float32), P)
    K = A.shape[1]
    bits = (2 ** np.arange(K)).astype(np.int64)
    return (((zW > 0).astype(np.int64)) * bits[None, :, None]).sum(1) % TABLE


def _query_mask(cm, W, A):
    codeW = _filter_codes(W, A)
    kh = W.shape[2] * W.shape[3]
    q = np.tile(np.asarray(cm, np.float32)[:, None], (1, kh)).reshape(-1)
    q = q / (np.sqrt((q * q).sum()) + 1e-8)
    Qv = np.concatenate([q, np.zeros(M_APPEND, np.float32)])
    zQ = np.einsum('lkd,d->lk', np.asarray(A, np.float32), Qv)
    K = A.shape[1]
    bits = (2 ** np.arange(K)).astype(np.int64)
    codeQ = (((zQ > 0).astype(np.int64)) * bits[None, :]).sum(1) % TABLE
    return (codeW == codeQ[:, None]).any(0)


def _collapse_A(A, in_ch):
    A = np.asarray(A, np.float32)
    return A[:, :, :in_ch * 25].reshape(A.shape[0], A.shape[1], in_ch, 25).sum(3)


def host_prep_fast(x, W1, A1, W2, A2, W3, A3, Wout, bout, active1):
    x = np.asarray(x, np.float32)
    W1 = np.asarray(W1, np.float32)
    W2 = np.asarray(W2, np.float32)
    W3 = np.asarray(W3, np.float32)
    Wout = np.asarray(Wout, np.float32)
    bout = np.asarray(bout, np.float32)
    a1 = len(active1)

    # ---- X staging: [108 = ic*36+iy, 36x * 256b] per core ----
    xsh = x.reshape(N_CORES, B_LOC, 3, 32, 32)
    xpad = np.zeros((N_CORES, B_LOC, 3, 36, 36), np.float32)
    xpad[:, :, :, 2:34, 2:34] = xsh
    xs_all = [xpad[c].transpose(1, 2, 3, 0).reshape(108, 36 * B_LOC).astype(BF)
              for c in range(N_CORES)]

    # ---- W1 banded: [108, 5 * 112]; lane = ypar*64 + yh*a1 + o ----
    W1b = np.zeros((108, 5 * 112), np.float32)
    for s in range(5):
        for ic in range(3):
            for y in range(32):
                yh, ypar = y // 2, y % 2
                for ky in range(5):
                    iy = y + ky
                    row = ic * 36 + iy
                    for o, oc in enumerate(active1):
                        W1b[row, s * 112 + ypar * 64 + yh * a1 + o] = \
                            W1[oc, ic, ky, s]

    # ---- W2 packed: [128, 5 * 104]; rows 32w + iy*a1+ic'; lane par*64+oyh*20+oc
    W2b = np.zeros((128, 5 * 104), np.float32)
    for iy in range(8):
        for icp, ic in enumerate(active1):
            r0 = iy * a1 + icp
            for s in range(5):
                for par in range(2):
                    for oyh in range(2):
                        ky = iy - (2 * oyh + par)
                        if 0 <= ky < 5:
                            for oc in range(20):
                                W2b[r0, s * 104 + par * 64 + oyh * 20 + oc] = \
                                    W2[oc, ic, ky, s]
    for w in range(1, 4):
        W2b[32 * w:32 * w + 8 * a1] = W2b[0:8 * a1]

    # ---- W3: [120, 5*40] rows iy*20+ic; lane par*20+oc (baseline layout) ----
    W3s = np.zeros((120, 5 * 40), np.float32)
    for iy in range(6):
        for ic in range(20):
            row = iy * 20 + ic
            for s in range(5):
                for par in range(2):
                    ky = iy - par
                    if 0 <= ky < 5:
                        for oc in range(20):
                            W3s[row, s * 40 + par * 20 + oc] = W3[oc, ic, ky, s]

    # ---- FC: Wouts [80, 40]: rows y3'*20+oc; col xq*10+co ----
    Wouts = np.zeros((80, 40), np.float32)
    for oc in range(20):
        for yq in range(4):
            for xq in range(4):
                for co in range(10):
                    Wouts[yq * 20 + oc, xq * 10 + co] = Wout[co, oc * 16 + yq * 4 + xq]

    # ---- hash constants ----
    At2 = _collapse_A(A2, 16)[:, :, active1]            # (2,6,a1)
    A2T = At2.transpose(2, 0, 1).reshape(a1, 12).copy()
    At3 = _collapse_A(A3, 20)
    A3T = At3.transpose(2, 0, 1).reshape(20, 30).copy()
    BW2 = np.zeros((12, 2), np.float32)
    for l in range(2):
        for k in range(6):
            BW2[l * 6 + k, l] = float((2 ** k) % TABLE)
    BW3 = np.zeros((30, 3), np.float32)
    for l in range(3):
        for k in range(10):
            BW3[l * 10 + k, l] = float((2 ** k) % TABLE)
    CW2 = _filter_codes(W2, A2).astype(np.float32)      # (2,20)
    CW3 = _filter_codes(W3, A3).astype(np.float32)      # (3,20)
    ONES2 = np.ones((2, 1), np.float32)
    ONES3 = np.ones((3, 1), np.float32)
    IND1 = np.zeros((16 * a1, a1), np.float32)
    for lane in range(16 * a1):
        IND1[lane, lane % a1] = 1.0
    IND2c = np.zeros((40, 20), np.float32)
    for lane in range(40):
        IND2c[lane, lane % 20] = 1.0
    EXP2 = np.zeros((20, 80), np.float32)
    for lane in range(80):
        EXP2[lane % 20, lane] = 1.0
    EXPD3 = np.zeros((20, 120), np.float32)
    for row in range(120):
        EXPD3[row % 20, row] = 1.0
    boutc = bout.reshape(10, 1).astype(np.float32)

    shared = dict(
        W1b=W1b.astype(BF), W2b=W2b.astype(BF), W3s=W3s.astype(BF),
        Wouts=Wouts.astype(BF),
        A2T=A2T, A3T=A3T, BW2=BW2, BW3=BW3, CW2=CW2, CW3=CW3,
        ONES2=ONES2, ONES3=ONES3, IND1=IND1, IND2c=IND2c, EXP2=EXP2,
        EXPD3=EXPD3, boutc=boutc,
        zeros=np.zeros((40, 16 * B_LOC), BF),
    )
    return shared, xs_all


# ---------------------------------------------------------------- device build
def build_nc_fast(a1):
    nc = bacc.Bacc("TRN2", target_bir_lowering=False, debug=False,
                   num_devices=N_CORES)
    npar1 = 16 * a1          # L1 lanes per parity (<= 64)
    nk2 = 8 * a1             # L2 K rows per w-slot (<= 32)

    ext = {}
    def ein(name, shape, dt):
        ext[name] = nc.dram_tensor(name, shape, dt, kind="ExternalInput")
        return ext[name]

    ein("X4", [108, 36 * B_LOC], bf16)
    ein("W1b", [108, 5 * 112], bf16)
    ein("W2b", [128, 5 * 104], bf16)
    ein("W3s", [120, 200], bf16)
    ein("Wouts", [80, 40], bf16)
    ein("A2T", [a1, 12], fp32)
    ein("A3T", [20, 30], fp32)
    ein("BW2", [12, 2], fp32)
    ein("BW3", [30, 3], fp32)
    ein("CW2", [2, 20], fp32)
    ein("CW3", [3, 20], fp32)
    ein("ONES2", [2, 1], fp32)
    ein("ONES3", [3, 1], fp32)
    ein("IND1", [16 * a1, a1], fp32)
    ein("IND2c", [40, 20], fp32)
    ein("EXP2", [20, 80], fp32)
    ein("EXPD3", [20, 120], fp32)
    ein("boutc", [10, 1], fp32)
    ein("zeros", [40, 16 * B_LOC], bf16)

    out_ext = nc.dram_tensor("out", [10, B_LOC], fp32, kind="ExternalOutput")

    with tile.TileContext(nc, num_cores=N_CORES) as tc:
        with (
            tc.tile_pool(name="const", bufs=1) as cpool,
            tc.tile_pool(name="work", bufs=1) as wpool,
            tc.tile_pool(name="psum", bufs=8, space="PSUM") as pp,
            tc.tile_pool(name="dram", bufs=1, space="DRAM") as dpool,
        ):
            # ------- X4 chunked load (sync queue) -------
            X4 = wpool.tile([108, 36 * B_LOC], bf16, tag="X4", name="X4")
            NCHUNK = 6
            xc = 36 // NCHUNK
            for ci in range(NCHUNK):
                c0 = ci * xc * B_LOC
                c1 = (ci + 1) * xc * B_LOC
                nc.sync.dma_start(X4[:, c0:c1], ext["X4"].ap()[:, c0:c1])

            # ------- constants (scalar queue; gpsimd reserved for collectives)
            def load(name, shape, dt, pool=cpool):
                t = pool.tile(shape, dt, tag=name, name=name)
                nc.scalar.dma_start(t[:], ext[name].ap())
                return t

            W1bb = load("W1b", [108, 5 * 112], bf16)
            W2bb = load("W2b", [128, 5 * 104], bf16)
            W3sb = load("W3s", [120, 200], bf16)
            Woutsb = load("Wouts", [80, 40], bf16)
            A2Tb = load("A2T", [a1, 12], fp32)
            A3Tb = load("A3T", [20, 30], fp32)
            BW2b = load("BW2", [12, 2], fp32)
            BW3b = load("BW3", [30, 3], fp32)
            CW2b = load("CW2", [2, 20], fp32)
            CW3b = load("CW3", [3, 20], fp32)
            ONES2b = load("ONES2", [2, 1], fp32)
            ONES3b = load("ONES3", [3, 1], fp32)
            IND1b = load("IND1", [16 * a1, a1], fp32)
            IND2cb = load("IND2c", [40, 20], fp32)
            EXP2b = load("EXP2", [20, 80], fp32)
            EXPD3b = load("EXPD3", [20, 120], fp32)
            boutb = load("boutc", [10, 1], fp32)
            one1 = cpool.tile([1, 1], fp32, tag="one1", name="one1")
            nc.vector.memset(one1[:], 1.0)

            # persistent tiles
            XPAB = wpool.tile([112, 16 * B_LOC], bf16, tag="XPAB", name="XPAB")
            XPB2 = wpool.tile([48, 16 * B_LOC], bf16, tag="XPB2", name="XPB2")
            P1u = wpool.tile([npar1, 16 * B_LOC], bf16, tag="P1u", name="P1u")
            P1 = wpool.tile([npar1, 16 * B_LOC], bf16, tag="P1", name="P1")
            S1acc = wpool.tile([npar1, 4], fp32, tag="S1acc", name="S1acc")
            S2 = wpool.tile([128, 20 * B_LOC], bf16, tag="S2", name="S2")
            XQAB = [wpool.tile([104, 8 * B_LOC], bf16, tag=f"XQAB_{w}",
                               name=f"XQAB_{w}") for w in range(4)]
            XQB2 = [wpool.tile([40, 8 * B_LOC], bf16, tag=f"XQB2_{w}",
                               name=f"XQB2_{w}") for w in range(4)]
            Q2u = [wpool.tile([40, 8 * B_LOC], bf16, tag=f"Q2u_{w}",
                              name=f"Q2u_{w}") for w in range(4)]
            S2acc = wpool.tile([40, 4], fp32, tag="S2acc", name="S2acc")
            S3 = [wpool.tile([120, 12 * B_LOC], bf16, tag=f"S3_{w}",
                             name=f"S3_{w}") for w in range(4)]
            XP3t = [wpool.tile([104, 4 * B_LOC], bf16, tag=f"XP3t_{t}",
                               name=f"XP3t_{t}") for t in range(2)]
            R3A = wpool.tile([80, 4 * B_LOC], bf16, tag="R3A", name="R3A")
            R3B = wpool.tile([80, 4 * B_LOC], bf16, tag="R3B", name="R3B")
            P3 = wpool.tile([80, 4 * B_LOC], bf16, tag="P3", name="P3")

            # x-halo zeroing only (y-halos handled by K-narrowing); vector is
            # idle this early
            nc.vector.memset(S2[:, 0:2 * B_LOC], 0.0)
            nc.vector.memset(S2[:, 18 * B_LOC:20 * B_LOC], 0.0)
            for w in range(4):
                nc.vector.memset(S3[w][:, 0:2 * B_LOC], 0.0)
                nc.vector.memset(S3[w][:, 10 * B_LOC:12 * B_LOC], 0.0)

            zerosb = ext["zeros"].ap()
            # y-halo rows: zero via DMA (cheap, off critical path)
            nc.sync.dma_start(S2[0:2 * a1, 2 * B_LOC:18 * B_LOC],
                              zerosb[0:2 * a1, :])
            nc.sync.dma_start(S2[96 + 6 * a1:96 + 8 * a1, 2 * B_LOC:18 * B_LOC],
                              zerosb[0:2 * a1, :])
            nc.scalar.dma_start(S3[0][0:40, 2 * B_LOC:10 * B_LOC],
                                zerosb[0:40, 0:8 * B_LOC])
            nc.scalar.dma_start(S3[3][80:120, 2 * B_LOC:10 * B_LOC],
                                zerosb[0:40, 0:8 * B_LOC])

            # PE warm-up burst (overlaps X4 DMA)
            for _ in range(2):
                wps = pp.tile([128, 320], fp32, tag="convps", name="convps")
                for ws in range(5):
                    nc.tensor.matmul(wps[0:112, :], X4[:, 0:112],
                                     X4[:, 0:320],
                                     start=(ws == 0), stop=(ws == 4))

            dps = pp.tile([128, 512], fp32, tag="convps", name="convps")

            def dummy(n=1):
                for _ in range(n):
                    nc.tensor.matmul(dps[0:108, :], W1bb[:, 0:108],
                                     W1bb[:, 0:512], start=True, stop=True)

            # ======================= LAYER 1 =======================
            # psum [112, 512]; lanes ypar*64 + yh*a1 + o; cols (x2, b256)
            for j in range(16):
                ps = pp.tile([128, 512], fp32, tag="convps", name="convps")
                for s in range(5):
                    nc.tensor.matmul(
                        ps[0:112, :],
                        W1bb[:, s * 112:(s + 1) * 112],
                        X4[:, (2 * j + s) * B_LOC:(2 * j + s + 2) * B_LOC],
                        start=(s == 0), stop=(s == 4))
                nc.vector.tensor_reduce(
                    out=XPAB[:, j * B_LOC:(j + 1) * B_LOC],
                    in_=ps[0:112, :].rearrange("p (x b) -> p b x", x=2),
                    axis=AX.X, op=AL.max)
                if j in (3, 7, 11, 15):
                    q = j // 4
                    qc = slice(q * 4 * B_LOC, (q + 1) * 4 * B_LOC)
                    eng = nc.sync if q % 2 == 0 else nc.scalar
                    eng.dma_start(XPB2[:, qc], XPAB[64:112, qc])
                    nc.vector.scalar_tensor_tensor(
                        out=P1[:, qc], in0=XPAB[0:npar1, qc], scalar=0.0,
                        in1=XPB2[0:npar1, qc], op0=AL.max, op1=AL.max,
                        accum_out=S1acc[:, q:q + 1])
                if j in (7, 15):
                    h = j // 8
                    hc = slice(h * 8 * B_LOC, (h + 1) * 8 * B_LOC)
                    # S2 staging for this x-half
                    for w in range(4):
                        ya, yb = max(0, 4 * w - 2), min(16, 4 * w + 6)
                        da = 32 * w + (ya - (4 * w - 2)) * a1
                        eng = nc.sync if w % 2 == 0 else nc.scalar
                        eng.dma_start(
                            S2[da:da + (yb - ya) * a1,
                               (2 + 8 * h) * B_LOC:(10 + 8 * h) * B_LOC],
                            P1[ya * a1:yb * a1, hc])

            # ---- s1 + AllReduce (emitted after L2 j2=1; PE never stalls) ----
            cc1_in = dpool.tile([a1, 1], fp32, tag="cc1_in", name="cc1_in")
            cc1_out = dpool.tile([a1, 1], fp32, tag="cc1_out", name="cc1_out")

            def s1_fold():
                s1ps = pp.tile([a1, 4], fp32, tag="convps", name="convps")
                nc.tensor.matmul(s1ps[:], IND1b[:], S1acc[:],
                                 start=True, stop=True)
                s1l2 = cpool.tile([a1, 4], fp32, tag="s1l2", name="s1l2")
                s1l = cpool.tile([a1, 1], fp32, tag="s1l", name="s1l")
                nc.scalar.activation(s1l2[:], s1ps[:], AF.Identity,
                                     accum_out=s1l[:])
                nc.gpsimd.dma_start(cc1_in[:], s1l[:])
                nc.gpsimd.collective_compute(
                    "AllReduce", AL.add,
                    replica_groups=[list(range(N_CORES))],
                    ins=[cc1_in.opt()], outs=[cc1_out.opt()])

            # ======================= LAYER 2 =======================
            # 4 w-slots row-packed; psum lanes par*64 + oyh*20 + oc
            # y-halo rows zeroed via zeros-DMA (32-align rule forbids
            # K-narrowed operand bases)
            def l2_j2(j2):
                pss2 = [pp.tile([128, 512], fp32, tag="convps",
                                name="convps") for _ in range(4)]
                for s in range(5):
                    for w in range(4):
                        nc.tensor.matmul(
                            pss2[w][0:104, :],
                            W2bb[32 * w:32 * w + nk2,
                                 s * 104:(s + 1) * 104],
                            S2[32 * w:32 * w + nk2,
                               (2 * j2 + s) * B_LOC:(2 * j2 + s + 2) * B_LOC],
                            start=(s == 0), stop=(s == 4),
                            tile_position=(32 * w, 0))
                for w in range(4):
                    nc.vector.tensor_reduce(
                        out=XQAB[w][:, j2 * B_LOC:(j2 + 1) * B_LOC],
                        in_=pss2[w][0:104, :].rearrange("p (x b) -> p b x", x=2),
                        axis=AX.X, op=AL.max)

            l2_j2(0)
            l2_j2(1)
            s1_fold()
            for j2 in range(2, 8):
                l2_j2(j2)

            # y-pool + relu + s2 accumulation (UNMASKED); relu split across
            # scalar (ACT, w odd) and vector (TS relu + reduce, w even)
            for w in range(4):
                eng = nc.sync if w % 2 == 0 else nc.scalar
                eng.dma_start(XQB2[w][:], XQAB[w][64:104, :])
                nc.vector.scalar_tensor_tensor(
                    out=Q2u[w][:], in0=XQAB[w][0:40, :], scalar=0.0,
                    in1=XQB2[w][:], op0=AL.max, op1=AL.max,
                    accum_out=S2acc[:, w:w + 1])

            # ---- s2 + AllReduce ----
            s2ps = pp.tile([20, 4], fp32, tag="convps", name="convps")
            nc.tensor.matmul(s2ps[:], IND2cb[:], S2acc[:], start=True, stop=True)
            s2loc = cpool.tile([20, 4], fp32, tag="s2loc", name="s2loc")
            s2l = cpool.tile([20, 1], fp32, tag="s2l", name="s2l")
            nc.scalar.activation(s2loc[:], s2ps[:], AF.Identity,
                                 accum_out=s2l[:])
            cc2_in = dpool.tile([20, 1], fp32, tag="cc2_in", name="cc2_in")
            cc2_out = dpool.tile([20, 1], fp32, tag="cc2_out", name="cc2_out")
            nc.sync.dma_start(cc2_in[:], s2l[:])
            nc.gpsimd.collective_compute(
                "AllReduce", AL.add, replica_groups=[list(range(N_CORES))],
                ins=[cc2_in.opt()], outs=[cc2_out.opt()])
            s1f = cpool.tile([a1, 1], fp32, tag="s1f", name="s1f")
            nc.gpsimd.dma_start(s1f[:], cc1_out[:])

            # ---- hash chain ----
            def hash_chain(v_sb, ATb, BWb, CWb, ONESb, LK, L, name,
                           warm=False):
                zps = pp.tile([LK, 1], fp32, tag="convps", name="convps")
                nc.tensor.matmul(zps[:], ATb[:], v_sb[:], start=True, stop=True)
                if warm:
                    dummy(2)
                bits = cpool.tile([LK, 1], fp32, tag=f"bits{name}", name=f"bits{name}")
                nc.vector.tensor_scalar(out=bits[:], in0=zps[:], scalar1=0.0,
                                        scalar2=None, op0=AL.is_gt)
                yps = pp.tile([L, 1], fp32, tag="convps", name="convps")
                nc.tensor.matmul(yps[:], BWb[:], bits[:], start=True, stop=True)
                if warm:
                    dummy(2)
                code = cpool.tile([L, 1], fp32, tag=f"code{name}", name=f"code{name}")
                nc.vector.tensor_copy(code[:], yps[:])
                ge = cpool.tile([L, 1], fp32, tag=f"ge{name}", name=f"ge{name}")
                for _ in range(2):
                    nc.vector.tensor_scalar(out=ge[:], in0=code[:],
                                            scalar1=24.5, scalar2=None,
                                            op0=AL.is_gt)
                    nc.vector.tensor_scalar(out=ge[:], in0=ge[:], scalar1=25.0,
                                            scalar2=None, op0=AL.mult)
                    nc.vector.tensor_tensor(out=code[:], in0=code[:],
                                            in1=ge[:], op=AL.subtract)
                eq = cpool.tile([L, 20], fp32, tag=f"eq{name}", name=f"eq{name}")
                nc.vector.tensor_tensor(out=eq[:], in0=CWb[:],
                                        in1=code[:].broadcast_to((L, 20)),
                                        op=AL.is_equal)
                cps = pp.tile([1, 20], fp32, tag="convps", name="convps")
                nc.tensor.matmul(cps[:], ONESb[:], eq[:], start=True, stop=True)
                mrow = cpool.tile([1, 20], fp32, tag=f"mrow{name}", name=f"mrow{name}")
                nc.vector.tensor_scalar(out=mrow[:], in0=cps[:], scalar1=0.5,
                                        scalar2=None, op0=AL.is_gt)
                mtp = pp.tile([20, 1], fp32, tag="convps", name="convps")
                nc.tensor.transpose(mtp[:], mrow[:], one1[:])
                mcol = cpool.tile([20, 1], fp32, tag=f"mcol{name}", name=f"mcol{name}")
                nc.vector.tensor_copy(mcol[:], mtp[:])
                return mcol

            # m2 from s1f; fold into W3 weights
            m2col = hash_chain(s1f, A2Tb, BW2b, CW2b, ONES2b, 12, 2, "m2",
                               warm=True)
            m2ps = pp.tile([120, 1], fp32, tag="convps", name="convps")
            nc.tensor.matmul(m2ps[:], EXPD3b[:], m2col[:], start=True, stop=True)
            dummy(4)
            m2r = cpool.tile([120, 1], fp32, tag="m2r", name="m2r")
            nc.vector.tensor_copy(m2r[:], m2ps[:])
            W3M = cpool.tile([120, 200], bf16, tag="W3M", name="W3M")
            nc.vector.tensor_scalar(out=W3M[:], in0=W3sb[:], scalar1=m2r[:],
                                    scalar2=None, op0=AL.mult)

            # ---- S3 staging (Q2u -> S3; no m2 dependency) ----
            _dq = 0
            for w3 in range(4):
                for w in range(max(0, w3 - 1), min(4, w3 + 2)):
                    da = (w - (w3 - 1)) * 40
                    eng = nc.sync if _dq % 2 == 0 else nc.scalar
                    _dq += 1
                    eng.dma_start(
                        S3[w3][da:da + 40, 2 * B_LOC:10 * B_LOC],
                        Q2u[w][:, :])
            s2f = cpool.tile([20, 1], fp32, tag="s2f", name="s2f")
            nc.gpsimd.dma_start(s2f[:], cc2_out[:])

            # ======================= LAYER 3 =======================

            def l3_t(t):
                pss3 = [pp.tile([128, 512], fp32, tag="convps",
                                name="convps") for _ in range(4)]
                for s in range(5):
                    for j3 in range(4):
                        for c in range(2):
                            nc.tensor.matmul(
                                pss3[j3][64 * c:64 * c + 40, :],
                                W3M[:, s * 40:(s + 1) * 40],
                                S3[2 * t + c][
                                    :, (2 * j3 + s) * B_LOC:
                                       (2 * j3 + s + 2) * B_LOC],
                                start=(s == 0), stop=(s == 4),
                                tile_position=(0, 64 * c))
                for j3 in range(4):
                    nc.vector.tensor_reduce(
                        out=XP3t[t][:, j3 * B_LOC:(j3 + 1) * B_LOC],
                        in_=pss3[j3][0:104, :].rearrange(
                            "p (x b) -> p b x", x=2),
                        axis=AX.X, op=AL.max)
                # per-t y-pool parity DMAs (overlap next t's matmuls)
                _dq3 = 0
                for c in range(2):
                    w3 = 2 * t + c
                    for par, dstt in ((0, R3A), (1, R3B)):
                        eng = nc.sync if _dq3 % 2 == 0 else nc.scalar
                        _dq3 += 1
                        eng.dma_start(
                            dstt[20 * w3:20 * w3 + 20, :],
                            XP3t[t][64 * c + 20 * par:64 * c + 20 * par + 20,
                                    :])

            l3_t(0)

            # ---- m3 chain interleaved with L3 t=1 (PE hops overlap) ----
            v3 = cpool.tile([20, 1], fp32, tag="v3", name="v3")
            nc.vector.tensor_tensor(out=v3[:], in0=s2f[:], in1=m2col[:],
                                    op=AL.mult)
            m3col = hash_chain(v3, A3Tb, BW3b, CW3b, ONES3b, 30, 3, "m3")
            m3ps = pp.tile([80, 1], fp32, tag="convps", name="convps")
            nc.tensor.matmul(m3ps[:], EXP2b[:], m3col[:], start=True, stop=True)
            m3exp = cpool.tile([80, 1], fp32, tag="m3exp", name="m3exp")
            nc.vector.tensor_copy(m3exp[:], m3ps[:])
            WoutM = cpool.tile([80, 40], bf16, tag="WoutM", name="WoutM")
            nc.vector.tensor_scalar(out=WoutM[:], in0=Woutsb[:],
                                    scalar1=m3exp[:], scalar2=None,
                                    op0=AL.mult)

            l3_t(1)

            # fused y-pool + relu -> P3
            nc.vector.scalar_tensor_tensor(
                out=P3[:], in0=R3A[:], scalar=0.0, in1=R3B[:],
                op0=AL.max, op1=AL.max)
            fcps = pp.tile([10, 256], fp32, tag="convps", name="convps")
            for xq in range(4):
                nc.tensor.matmul(fcps[:],
                                 WoutM[:, xq * 10:(xq + 1) * 10],
                                 P3[:, xq * B_LOC:(xq + 1) * B_LOC],
                                 start=(xq == 0), stop=(xq == 3))
            out_sb = cpool.tile([10, 256], fp32, tag="out_sb", name="out_sb")
            nc.scalar.activation(out_sb[:], fcps[:], AF.Identity, bias=boutb[:])
            nc.sync.dma_start(out_ext.ap(), out_sb[:])

    nc.compile()
    return nc


# ======== baseline fallback (a1 > 4) ========
def _host_prep_slow(x, W1, A1, W2, A2, W3, A3, Wout, bout, force_masks=False):
    """Build all per-core / shared device tensors. Returns (shared, per_core list)."""
    x = np.asarray(x, np.float32)
    W1 = np.asarray(W1, np.float32)
    W2 = np.asarray(W2, np.float32)
    W3 = np.asarray(W3, np.float32)
    Wout = np.asarray(Wout, np.float32)
    bout = np.asarray(bout, np.float32)

    # ---- layer-1 mask from full-batch channel means (host) ----
    m1 = _query_mask(x.mean((0, 2, 3)), W1, A1).astype(np.float32)  # (16,)
    if force_masks:
        m1 = np.ones(16, np.float32)

    # ---- X staging: [128, 2*256*36] bf16 ----
    # quadrant r rows (iy8, ic3) iy-major; windows w = c*4 + r at wsel=c
    xs_all = []
    xsh = x.reshape(N_CORES, B_LOC, 3, 32, 32)
    xpad = np.zeros((N_CORES, B_LOC, 3, 36, 36), np.float32)
    xpad[:, :, :, 2:34, 2:34] = xsh
    for core in range(N_CORES):
        X4 = np.zeros((128, 2 * B_LOC * 36), np.float32)
        for w in range(8):
            r, c = w % 4, w // 4
            for iy in range(8):
                gy = 4 * w - 2 + iy + 2  # row in padded (0..35)
                for ic in range(3):
                    row = 32 * r + iy * 3 + ic
                    X4[row, c * (B_LOC * 36):(c + 1) * (B_LOC * 36)] = \
                        xpad[core, :, ic, gy, :].reshape(-1)
        xs_all.append(X4.astype(BF))

    # ---- weight stagings ----
    # L1: W1s [128, 5*64]: quadrant r rows (iy8,ic3); col (s*64 + par*32+oyh*16+oc)
    W1s = np.zeros((128, 5 * 64), np.float32)
    for r in range(4):
        for iy in range(8):
            for ic in range(3):
                row = 32 * r + iy * 3 + ic
                for s in range(5):
                    for par in range(2):
                        for oyh in range(2):
                            oy = 2 * oyh + par
                            ky = iy - oy
                            if 0 <= ky < 5:
                                for oc in range(16):
                                    W1s[row, s * 64 + par * 32 + oyh * 16 + oc] = \
                                        W1[oc, ic, ky, s]
    # L2: W2s [128, 5*80]: rows (iy8, ic16) iy-major; lane par*40+oyh*20+oc
    W2s = np.zeros((128, 5 * 80), np.float32)
    for iy in range(8):
        for ic in range(16):
            row = iy * 16 + ic
            for s in range(5):
                for par in range(2):
                    for oyh in range(2):
                        oy = 2 * oyh + par
                        ky = iy - oy
                        if 0 <= ky < 5:
                            for oc in range(20):
                                W2s[row, s * 80 + par * 40 + oyh * 20 + oc] = \
                                    W2[oc, ic, ky, s]
    # L3: W3s [120, 5*40]: rows (iy6, ic20) iy-major; lane par*20+oc
    W3s = np.zeros((120, 5 * 40), np.float32)
    for iy in range(6):
        for ic in range(20):
            row = iy * 20 + ic
            for s in range(5):
                for par in range(2):
                    ky = iy - par
                    if 0 <= ky < 5:
                        for oc in range(20):
                            W3s[row, s * 40 + par * 20 + oc] = W3[oc, ic, ky, s]
    # FC: Wouts [80, 4*10]: rows (y'4, oc20) = y'*20+oc; col x'*10+co
    Wouts = np.zeros((80, 40), np.float32)
    for oc in range(20):
        for yq in range(4):
            for xq in range(4):
                for co in range(10):
                    Wouts[yq * 20 + oc, xq * 10 + co] = Wout[co, oc * 16 + yq * 4 + xq]

    # ---- hash constants ----
    At2 = _collapse_A(A2, 16) * m1[None, None, :]       # (2,6,16), m1 folded
    A2T = At2.transpose(2, 0, 1).reshape(16, 12).copy() # rows c, cols (l,k)
    At3 = _collapse_A(A3, 20)                           # (3,10,20)
    A3T = At3.transpose(2, 0, 1).reshape(20, 30).copy()
    BW2 = np.zeros((12, 2), np.float32)
    for l in range(2):
        for k in range(6):
            BW2[l * 6 + k, l] = float((2 ** k) % TABLE)
    BW3 = np.zeros((30, 3), np.float32)
    for l in range(3):
        for k in range(10):
            BW3[l * 10 + k, l] = float((2 ** k) % TABLE)
    CW2 = _filter_codes(W2, A2).astype(np.float32)      # (2,20)
    CW3 = _filter_codes(W3, A3).astype(np.float32)      # (3,20)
    ONES2 = np.ones((2, 1), np.float32)
    ONES3 = np.ones((3, 1), np.float32)
    # P1 lanes: r4*32 + oyh2*16 + oc16 -> oc = lane % 16
    IND1 = np.zeros((128, 16), np.float32)
    for lane in range(128):
        IND1[lane, lane % 16] = 1.0
    # Q2 lanes: q2*40 + oyh2*20 + oc20 -> oc = lane % 20
    IND2 = np.zeros((80, 20), np.float32)
    for lane in range(80):
        IND2[lane, lane % 20] = 1.0
    # EXP2 [20,80]: m[20] -> 80 lanes with oc = lane % 20 (Q2 and P3 layouts)
    EXP2 = np.zeros((20, 80), np.float32)
    for lane in range(80):
        EXP2[lane % 20, lane] = 1.0
    # m1 expanded to P1 lanes (oc = lane % 16)
    m1exp = np.tile(m1, 8)[:, None].astype(np.float32)  # (128,1)
    # mean scales folded: reference cm2 = sum(h1)/(2048*16*16); A2T already has m1.
    # sign-invariant to positive scale -> no normalization needed.
    boutc = bout.reshape(10, 1).astype(np.float32)

    if force_masks:
        BW2[:] = 0.0; BW3[:] = 0.0; CW2 = np.zeros_like(CW2); CW3 = np.zeros_like(CW3)
    shared = dict(
        W1s=W1s.astype(BF), W2s=W2s.astype(BF), W3s=W3s.astype(BF),
        Wouts=Wouts.astype(BF),
        A2T=A2T, A3T=A3T, BW2=BW2, BW3=BW3, CW2=CW2, CW3=CW3,
        ONES2=ONES2, ONES3=ONES3, IND1=IND1, IND2=IND2, EXP2=EXP2,
        m1exp=m1exp, boutc=boutc,
        zeros=np.zeros((128, 5120), BF),
    )
    return shared, xs_all



def build_nc_slow(debug=False):
    nc = bacc.Bacc("TRN2", target_bir_lowering=False, debug=False,
                   num_devices=N_CORES)

    ext = {}
    def ein(name, shape, dt):
        ext[name] = nc.dram_tensor(name, shape, dt, kind="ExternalInput")
        return ext[name]

    ein("X4", [128, 2 * B_LOC * 36], bf16)
    ein("W1s", [128, 320], bf16)
    ein("W2s", [128, 400], bf16)
    ein("W3s", [120, 200], bf16)
    ein("Wouts", [80, 40], bf16)
    ein("A2T", [16, 12], fp32)
    ein("A3T", [20, 30], fp32)
    ein("BW2", [12, 2], fp32)
    ein("BW3", [30, 3], fp32)
    ein("CW2", [2, 20], fp32)
    ein("CW3", [3, 20], fp32)
    ein("ONES2", [2, 1], fp32)
    ein("ONES3", [3, 1], fp32)
    ein("IND1", [128, 16], fp32)
    ein("IND2", [80, 20], fp32)
    ein("EXP2", [20, 80], fp32)
    ein("m1exp", [128, 1], fp32)
    ein("boutc", [10, 1], fp32)
    ein("zeros", [128, 5120], bf16)

    out_ext = nc.dram_tensor("out", [10, B_LOC], fp32, kind="ExternalOutput")
    dbg = {}
    if debug:
        for nm, shape in [("d_XP1_0", [128, 4096]), ("d_P1_0", [128, 4096]),
                          ("d_P1_1", [128, 4096]),
                           ("d_XP2_0", [128, 2048]),
                          ("d_Q2_0", [80, 2048]), ("d_Q2_1", [80, 2048]),
                          ("d_S3_1", [120, 3072]),
                          ("d_XP3_0", [128, 1024]), ("d_P3", [80, 1024]),
                          ("d_m", [80, 4])]:
            dbg[nm] = nc.dram_tensor(nm, shape, fp32, kind="ExternalOutput")

    with tile.TileContext(nc, num_cores=N_CORES) as tc:
        with (
            tc.tile_pool(name="const", bufs=1) as cpool,
            tc.tile_pool(name="work", bufs=1) as wpool,
            tc.tile_pool(name="scratch", bufs=(3 if debug else 6)) as spool,
            tc.tile_pool(name="dbgpool", bufs=1) as dbgpool,
            tc.tile_pool(name="psum", bufs=8, space="PSUM") as pp,
            tc.tile_pool(name="dram", bufs=1, space="DRAM") as dpool,
        ):
            # ------- X4 load first (largest, gates L1) -------
            X4g = wpool.tile([128, 2 * B_LOC * 36], bf16, tag="X4",
                             name="X4")
            nc.sync.dma_start(X4g[:], ext["X4"].ap())

            # ------- load constants (gpsimd queue; sync stays free) -------
            def load(name, shape, dt, pool=cpool):
                t = pool.tile(shape, dt, tag=name)
                nc.gpsimd.dma_start(t[:], ext[name].ap())
                return t

            W1sb = load("W1s", [128, 320], bf16)
            W2sb = load("W2s", [128, 400], bf16)
            W3sb = load("W3s", [120, 200], bf16)
            Woutsb = load("Wouts", [80, 40], bf16)
            A2Tb = load("A2T", [16, 12], fp32)
            A3Tb = load("A3T", [20, 30], fp32)
            BW2b = load("BW2", [12, 2], fp32)
            BW3b = load("BW3", [30, 3], fp32)
            CW2b = load("CW2", [2, 20], fp32)
            CW3b = load("CW3", [3, 20], fp32)
            ONES2b = load("ONES2", [2, 1], fp32)
            ONES3b = load("ONES3", [3, 1], fp32)
            IND1b = load("IND1", [128, 16], fp32)
            IND2b = load("IND2", [80, 20], fp32)
            EXP2b = load("EXP2", [20, 80], fp32)
            m1expb = load("m1exp", [128, 1], fp32)
            boutb = load("boutc", [10, 1], fp32)
            one1 = cpool.tile([1, 1], fp32, tag="one1", name="one1")
            nc.vector.memset(one1[:], 1.0)

            # PE warm-up burst (overlaps the X4 DMA; warms the HAM clock)
            for _ in range(3):
                wps = pp.tile([128, 320], fp32, tag="convps", name="convps")
                for ws in range(5):
                    nc.tensor.matmul(wps[:], W1sb[:, 0:128], W1sb[:],
                                     start=(ws == 0), stop=(ws == 4))

            zext = ext["zeros"].ap()

            def dump(nm, t):
                if not debug:
                    return
                f = dbgpool.tile(list(t.shape), bf16, tag="dumpf", name="dumpf")
                nc.vector.tensor_copy(f[:], t[:])
                nc.gpsimd.dma_start(dbg[nm].ap(), f[:])

            # persistent pooled-activation tiles (outer pool)
            P1 = [wpool.tile([128, 16 * 256], bf16, tag=f"P1_{c}", name=f"P1_{c}")
                  for c in range(2)]
            S1acc = wpool.tile([128, 2], fp32, tag="S1acc", name="S1acc")
            Q2 = [wpool.tile([80, 8 * 256], bf16, tag=f"Q2_{h}", name=f"Q2_{h}") for h in range(2)]
            S2 = [wpool.tile([128, 20 * B_LOC], bf16, tag=f"S2_{w}",
                             name="S2") for w in range(4)]
            S3 = [wpool.tile([120, 12 * B_LOC], bf16, tag=f"S3_{w}",
                             name="S3") for w in range(4)]
            # pad DMAs issued up-front (depend on nothing)
            for w in range(4):
                dpad = S2[w].rearrange("p (blk q) -> p blk q", q=512)
                nc.sync.dma_start(dpad[:, 0:10:9, :],
                                  zext[0:128, 0:1024].rearrange(
                                      "p (a q) -> p a q", q=512))
                dpad3 = S3[w].rearrange("p (blk q) -> p blk q", q=512)
                nc.sync.dma_start(dpad3[:, 0:6:5, :],
                                  zext[0:120, 0:1024].rearrange(
                                      "p (a q) -> p a q", q=512))
            nc.sync.dma_start(S2[0][0:32, 2 * B_LOC:18 * B_LOC],
                              zext[0:32, 0:16 * B_LOC])
            nc.sync.dma_start(S2[3][96:128, 2 * B_LOC:18 * B_LOC],
                              zext[0:32, 0:16 * B_LOC])
            nc.sync.dma_start(S3[0][0:40, 2 * B_LOC:10 * B_LOC],
                              zext[0:40, 0:8 * B_LOC])
            nc.sync.dma_start(S3[3][80:120, 2 * B_LOC:10 * B_LOC],
                              zext[0:40, 0:8 * B_LOC])
            S2acc = wpool.tile([80, 2], fp32, tag="S2acc", name="S2acc")
            P3 = wpool.tile([80, 4 * 256], bf16, tag="P3", name="P3")

            # ======================= LAYER 1 =======================
            with tc.tile_pool(name="l1big", bufs=1) as l1pool:
                X4 = X4g
                XP1 = [l1pool.tile([128, 16 * 256], bf16, tag=f"XP1_{r}", name=f"XP1_{r}")
                       for r in range(4)]

                tile_count = 0
                v = X4.rearrange("p (w b x) -> p w b x", w=2, b=B_LOC)
                for j in range(16):
                    pss = [pp.tile([128, 512], fp32, tag="convps",
                                   name="convps") for _ in range(4)]
                    for s in range(5):
                        for c in range(2):
                            for r in range(4):
                                nc.tensor.matmul(
                                    pss[r][64 * c:64 * c + 64, :],
                                    W1sb[32 * r:32 * r + 24,
                                         s * 64:(s + 1) * 64],
                                    v[32 * r:32 * r + 24, c, :,
                                      2 * j + s:2 * j + s + 2],
                                    start=(s == 0), stop=(s == 4),
                                    tile_position=(32 * r, 64 * c))
                    for r in range(4):
                        ps = pss[r]
                        use_act = (tile_count % 20) < int(ACT_EVAC_FRAC * 20)
                        tile_count += 1
                        if use_act:
                            sc = spool.tile([128, 512], bf16, tag="evac",
                                            name="evac")
                            nc.scalar.activation(sc[:], ps[:], AF.Copy)
                            vv = sc.rearrange("p (b x) -> p b x", x=2)
                            nc.vector.tensor_tensor(
                                out=XP1[r][:, j * 256:j * 256 + 256],
                                in0=vv[:, :, 0], in1=vv[:, :, 1], op=AL.max)
                        else:
                            nc.vector.tensor_reduce(
                                out=XP1[r][:, j * 256:j * 256 + 256],
                                in_=ps.rearrange("p (b x) -> p b x", x=2),
                                axis=AX.X, op=AL.max)

                # y-pool: parity-split DMAs then TT max -> P1
                P1A = [l1pool.tile([128, 16 * 256], bf16, tag=f"P1A_{c}", name=f"P1A_{c}")
                       for c in range(2)]
                P1B = [l1pool.tile([128, 16 * 256], bf16, tag=f"P1B_{c}", name=f"P1B_{c}")
                       for c in range(2)]
                # P1 lanes: 32r + 16oyh + oc (y' = 8c + 2r + oyh)
                _dq = 0
                for r in range(4):
                    for c in range(2):
                        for par, dstt in ((0, P1A), (1, P1B)):
                            eng = nc.sync if _dq % 2 == 0 else nc.gpsimd
                            _dq += 1
                            eng.dma_start(
                                dstt[c][32 * r:32 * r + 32, :],
                                XP1[r][64 * c + 32 * par:
                                       64 * c + 32 * par + 32, :])
                dump("d_XP1_0", XP1[0])
                for c in range(2):
                    nc.vector.tensor_tensor(out=P1[c][:], in0=P1A[c][:],
                                            in1=P1B[c][:], op=AL.max)
                    nc.scalar.activation(P1[c][:], P1[c][:], AF.Relu,
                                         scale=m1expb[:],
                                         accum_out=S1acc[:, c:c + 1])
                dump("d_P1_0", P1[0])
                dump("d_P1_1", P1[1])

            # ---- s1 partial + AllReduce ----
            s1ps = pp.tile([16, 2], fp32, tag="convps", name="convps")
            nc.tensor.matmul(s1ps[:], IND1b[:], S1acc[:], start=True, stop=True)
            s1loc = cpool.tile([16, 2], fp32, tag="s1loc", name="s1loc")
            nc.vector.tensor_copy(s1loc[:], s1ps[:])
            s1l = cpool.tile([16, 1], fp32, tag="s1l", name="s1l")
            nc.vector.tensor_tensor(out=s1l[:], in0=s1loc[:, 0:1],
                                    in1=s1loc[:, 1:2], op=AL.add)
            cc1_in = dpool.tile([16, 1], fp32)
            cc1_out = dpool.tile([16, 1], fp32)
            nc.sync.dma_start(cc1_in[:], s1l[:])
            nc.gpsimd.collective_compute(
                "AllReduce", AL.add, replica_groups=[list(range(N_CORES))],
                ins=[cc1_in.opt()], outs=[cc1_out.opt()])
            s1f = cpool.tile([16, 1], fp32, tag="s1f", name="s1f")
            nc.sync.dma_start(s1f[:], cc1_out[:])

            # ---- hash chain ----
            def hash_chain(v_sb, ATb, BWb, CWb, ONESb, LK, L, name):
                zps = pp.tile([LK, 1], fp32, tag="convps", name="convps")
                nc.tensor.matmul(zps[:], ATb[:], v_sb[:], start=True, stop=True)
                bits = cpool.tile([LK, 1], fp32, tag=f"bits{name}", name=f"bits{name}")
                nc.vector.tensor_scalar(out=bits[:], in0=zps[:], scalar1=0.0,
                                        scalar2=None, op0=AL.is_gt)
                yps = pp.tile([L, 1], fp32, tag="convps", name="convps")
                nc.tensor.matmul(yps[:], BWb[:], bits[:], start=True, stop=True)
                code = cpool.tile([L, 1], fp32, tag=f"code{name}", name=f"code{name}")
                nc.vector.tensor_copy(code[:], yps[:])
                ge = cpool.tile([L, 1], fp32, tag=f"ge{name}", name=f"ge{name}")
                for _ in range(2):
                    nc.vector.tensor_scalar(out=ge[:], in0=code[:],
                                            scalar1=24.5, scalar2=None,
                                            op0=AL.is_gt)
                    nc.vector.tensor_scalar(out=ge[:], in0=ge[:], scalar1=25.0,
                                            scalar2=None, op0=AL.mult)
                    nc.vector.tensor_tensor(out=code[:], in0=code[:],
                                            in1=ge[:], op=AL.subtract)
                eq = cpool.tile([L, 20], fp32, tag=f"eq{name}", name=f"eq{name}")
                nc.vector.tensor_tensor(out=eq[:], in0=CWb[:],
                                        in1=code[:].broadcast_to((L, 20)),
                                        op=AL.is_equal)
                cps = pp.tile([1, 20], fp32, tag="convps", name="convps")
                nc.tensor.matmul(cps[:], ONESb[:], eq[:], start=True, stop=True)
                mrow = cpool.tile([1, 20], fp32, tag=f"mrow{name}", name=f"mrow{name}")
                nc.vector.tensor_scalar(out=mrow[:], in0=cps[:], scalar1=0.5,
                                        scalar2=None, op0=AL.is_gt)
                mtp = pp.tile([20, 1], fp32, tag="convps", name="convps")
                nc.tensor.transpose(mtp[:], mrow[:], one1[:])
                mcol = cpool.tile([20, 1], fp32, tag=f"mcol{name}", name=f"mcol{name}")
                nc.vector.tensor_copy(mcol[:], mtp[:])
                return mcol

            # ======================= LAYER 2 =======================
            with tc.tile_pool(name="l2big", bufs=1) as l2pool:
                # S2_w [128=(iy8,ic16), (xp20, b256)]
                # S2 valid-row staging (pads already zeroed up-front)
                for w in range(4):
                    y0, y1 = max(0, 4 * w - 2), min(16, 4 * w + 6)
                    iy0 = y0 - (4 * w - 2)
                    for ch in range(2):
                        ya, yb = max(y0, 8 * ch), min(y1, 8 * ch + 8)
                        if ya >= yb:
                            continue
                        la = 16 * (ya % 8)
                        da = 16 * (iy0 + (ya - y0))
                        nc.sync.dma_start(
                            S2[w][da:da + 16 * (yb - ya),
                                  2 * B_LOC:18 * B_LOC],
                            P1[ch][la:la + 16 * (yb - ya), :])

                XP2 = [l2pool.tile([128, 8 * 256], bf16, tag=f"XP2_{w}", name=f"XP2_{w}")
                       for w in range(4)]
                tile_count = 0
                for w in range(4):
                    rhsv = S2[w].rearrange("p (xp b) -> p b xp", b=B_LOC)
                    for jg in range(2):
                        pss = [pp.tile([128, 512], fp32, tag="convps",
                                       name="convps") for _ in range(4)]
                        for srt in range(5):
                            for jj in range(4):
                                j = 4 * jg + jj
                                nc.tensor.matmul(
                                    pss[jj][0:80, :],
                                    W2sb[:, srt * 80:(srt + 1) * 80],
                                    rhsv[:, :, 2 * j + srt:2 * j + srt + 2],
                                    start=(srt == 0), stop=(srt == 4))
                        for jj in range(4):
                            j = 4 * jg + jj
                            ps = pss[jj]
                            use_act = (tile_count % 20) < int(ACT_EVAC_FRAC * 20)
                            tile_count += 1
                            if use_act:
                                sc = spool.tile([128, 512], bf16, tag="evac",
                                                name="evac")
                                nc.scalar.activation(sc[0:80, :], ps[0:80, :],
                                                     AF.Copy)
                                vv = sc.rearrange("p (b x) -> p b x", x=2)
                                nc.vector.tensor_tensor(
                                    out=XP2[w][0:80, j * 256:j * 256 + 256],
                                    in0=vv[0:80, :, 0], in1=vv[0:80, :, 1],
                                    op=AL.max)
                            else:
                                nc.vector.tensor_reduce(
                                    out=XP2[w][0:80, j * 256:j * 256 + 256],
                                    in_=ps[0:80, :].rearrange(
                                        "p (b x) -> p b x", x=2),
                                    axis=AX.X, op=AL.max)

                m2col = hash_chain(s1f, A2Tb, BW2b, CW2b, ONES2b,
                                   12, 2, "m2")
                m2ps = pp.tile([80, 1], fp32, tag="convps", name="convps")
                nc.tensor.matmul(m2ps[:], EXP2b[:], m2col[:],
                                 start=True, stop=True)
                m2exp = cpool.tile([80, 1], fp32, tag="m2exp", name="m2exp")
                nc.vector.tensor_copy(m2exp[:], m2ps[:])

                Q2A = [l2pool.tile([80, 8 * 256], bf16, tag=f"Q2A_{h}",
                                   name="Q2A") for h in range(2)]
                Q2B = [l2pool.tile([80, 8 * 256], bf16, tag=f"Q2B_{h}",
                                   name="Q2B") for h in range(2)]
                # Q2 lanes: 40q + 20oyh + oc (y' = 4h + 2q + oyh, q = w % 2)
                _dq = 0
                for w in range(4):


# revision 18
# speedup vs baseline: 1.0852x; 1.0852x over previous
"""ALSHConvNet forward on 8 TRN2 NeuronCores — fast path (a1 <= 4).

Key ideas vs baseline:
- L1 as banded-K matmul: K=108 (3ic x 36iy), M=112 (y-parity split at
  partition 64), 80 streaming passes total (was 640 instructions).
- m1 (host-known) compacts conv2's input channels: K=8*a1<=32 per
  y-window, 4 windows row-packed via tile_position -> 4x fewer passes.
- Q2/s2 kept UNMASKED; m2 folded into conv3 weights (W3M) so the L2->L3
  data path never waits on the s1 AllReduce result for activations.
- x-outer staging layouts; pooling via cross-partition DVE ops (no DMA
  round-trips except L3 parity + staging).
"""
import sys
sys.path.insert(0, '/opt/trn_rl_repo')

import numpy as np
import ml_dtypes

import concourse.bacc as bacc
import concourse.mybir as mybir
import concourse.tile as tile
from concourse.bass_utils import run_bass_kernel_spmd

N_CORES = 8
B_LOC = 256
fp32 = mybir.dt.float32
bf16 = mybir.dt.bfloat16
AL = mybir.AluOpType
AF = mybir.ActivationFunctionType
AX = mybir.AxisListType
BF = ml_dtypes.bfloat16

U = 0.9999
M_APPEND = 3
TABLE = 25


# ---------------------------------------------------------------- host math
def _filter_codes(W, A):
    out_ch = W.shape[0]
    Wf = np.asarray(W, np.float32).reshape(out_ch, -1)
    norms = np.sqrt((Wf * Wf).sum(1))
    Wp = Wf * (U / norms.max())
    n2 = (Wp * Wp).sum(1)
    terms = np.stack([0.5 - n2 ** (2 ** i) for i in range(M_APPEND)], 1)
    P = np.concatenate([Wp, terms], 1).astype(np.float32)
    zW = np.einsum('lkd,nd->lkn', np.asarray(A, np.float32), P)
    K = A.shape[1]
    bits = (2 ** np.arange(K)).astype(np.int64)
    return (((zW > 0).astype(np.int64)) * bits[None, :, None]).sum(1) % TABLE


def _query_mask(cm, W, A):
    codeW = _filter_codes(W, A)
    kh = W.shape[2] * W.shape[3]
    q = np.tile(np.asarray(cm, np.float32)[:, None], (1, kh)).reshape(-1)
    q = q / (np.sqrt((q * q).sum()) + 1e-8)
    Qv = np.concatenate([q, np.zeros(M_APPEND, np.float32)])
    zQ = np.einsum('lkd,d->lk', np.asarray(A, np.float32), Qv)
    K = A.shape[1]
    bits = (2 ** np.arange(K)).astype(np.int64)
    codeQ = (((zQ > 0).astype(np.int64)) * bits[None, :]).sum(1) % TABLE
    return (codeW == codeQ[:, None]).any(0)


def _collapse_A(A, in_ch):
    A = np.asarray(A, np.float32)
    return A[:, :, :in_ch * 25].reshape(A.shape[0], A.shape[1], in_ch, 25).sum(3)


def host_prep_fast(x, W1, A1, W2, A2, W3, A3, Wout, bout, active1):
    x = np.asarray(x, np.float32)
    W1 = np.asarray(W1, np.float32)
    W2 = np.asarray(W2, np.float32)
    W3 = np.asarray(W3, np.float32)
    Wout = np.asarray(Wout, np.float32)
    bout = np.asarray(bout, np.float32)
    a1 = len(active1)

    # ---- X staging: [108 = ic*36+iy, 36x * 256b] per core ----
    xsh = x.reshape(N_CORES, B_LOC, 3, 32, 32)
    xpad = np.zeros((N_CORES, B_LOC, 3, 36, 36), np.float32)
    xpad[:, :, :, 2:34, 2:34] = xsh
    xs_all = [xpad[c].transpose(1, 2, 3, 0).reshape(108, 36 * B_LOC).astype(BF)
              for c in range(N_CORES)]

    # ---- W1 banded: [108, 5 * 112]; lane = ypar*64 + yh*a1 + o ----
    W1b = np.zeros((108, 5 * 112), np.float32)
    for s in range(5):
        for ic in range(3):
            for y in range(32):
                yh, ypar = y // 2, y % 2
                for ky in range(5):
                    iy = y + ky
                    row = ic * 36 + iy
                    for o, oc in enumerate(active1):
                        W1b[row, s * 112 + ypar * 64 + yh * a1 + o] = \
                            W1[oc, ic, ky, s]

    # ---- W2 packed: [128, 5 * 104]; rows 32w + iy*a1+ic'; lane par*64+oyh*20+oc
    W2b = np.zeros((128, 5 * 104), np.float32)
    for iy in range(8):
        for icp, ic in enumerate(active1):
            r0 = iy * a1 + icp
            for s in range(5):
                for par in range(2):
                    for oyh in range(2):
                        ky = iy - (2 * oyh + par)
                        if 0 <= ky < 5:
                            for oc in range(20):
                                W2b[r0, s * 104 + par * 64 + oyh * 20 + oc] = \
                                    W2[oc, ic, ky, s]
    for w in range(1, 4):
        W2b[32 * w:32 * w + 8 * a1] = W2b[0:8 * a1]

    # ---- W3: [120, 5*40] rows iy*20+ic; lane par*20+oc (baseline layout) ----
    W3s = np.zeros((120, 5 * 40), np.float32)
    for iy in range(6):
        for ic in range(20):
            row = iy * 20 + ic
            for s in range(5):
                for par in range(2):
                    ky = iy - par
                    if 0 <= ky < 5:
                        for oc in range(20):
                            W3s[row, s * 40 + par * 20 + oc] = W3[oc, ic, ky, s]

    # ---- FC: Wouts [80, 40]: rows y3'*20+oc; col xq*10+co ----
    Wouts = np.zeros((80, 40), np.float32)
    for oc in range(20):
        for yq in range(4):
            for xq in range(4):
                for co in range(10):
                    Wouts[yq * 20 + oc, xq * 10 + co] = Wout[co, oc * 16 + yq * 4 + xq]

    # ---- hash constants ----
    At2 = _collapse_A(A2, 16)[:, :, active1]            # (2,6,a1)
    A2T = At2.transpose(2, 0, 1).reshape(a1, 12).copy()
    At3 = _collapse_A(A3, 20)
    A3T = At3.transpose(2, 0, 1).reshape(20, 30).copy()
    BW2 = np.zeros((12, 2), np.float32)
    for l in range(2):
        for k in range(6):
            BW2[l * 6 + k, l] = float((2 ** k) % TABLE)
    BW3 = np.zeros((30, 3), np.float32)
    for l in range(3):
        for k in range(10):
            BW3[l * 10 + k, l] = float((2 ** k) % TABLE)
    CW2 = _filter_codes(W2, A2).astype(np.float32)      # (2,20)
    CW3 = _filter_codes(W3, A3).astype(np.float32)      # (3,20)
    ONES2 = np.ones((2, 1), np.float32)
    ONES3 = np.ones((3, 1), np.float32)
    IND1 = np.zeros((16 * a1, a1), np.float32)
    for lane in range(16 * a1):
        IND1[lane, lane % a1] = 1.0
    IND2c = np.zeros((40, 20), np.float32)
    for lane in range(40):
        IND2c[lane, lane % 20] = 1.0
    EXP2 = np.zeros((20, 80), np.float32)
    for lane in range(80):
        EXP2[lane % 20, lane] = 1.0
    EXPD3 = np.zeros((20, 120), np.float32)
    for row in range(120):
        EXPD3[row % 20, row] = 1.0
    boutc = bout.reshape(10, 1).astype(np.float32)

    shared = dict(
        W1b=W1b.astype(BF), W2b=W2b.astype(BF), W3s=W3s.astype(BF),
        Wouts=Wouts.astype(BF),
        A2T=A2T, A3T=A3T, BW2=BW2, BW3=BW3, CW2=CW2, CW3=CW3,
        ONES2=ONES2, ONES3=ONES3, IND1=IND1, IND2c=IND2c, EXP2=EXP2,
        EXPD3=EXPD3, boutc=boutc,
        zeros=np.zeros((40, 16 * B_LOC), BF),
    )
    return shared, xs_all


# ---------------------------------------------------------------- device build
def build_nc_fast(a1):
    nc = bacc.Bacc("TRN2", target_bir_lowering=False, debug=False,
                   num_devices=N_CORES)
    npar1 = 16 * a1          # L1 lanes per parity (<= 64)
    nk2 = 8 * a1             # L2 K rows per w-slot (<= 32)

    ext = {}
    def ein(name, shape, dt):
        ext[name] = nc.dram_tensor(name, shape, dt, kind="ExternalInput")
        return ext[name]

    ein("X4", [108, 36 * B_LOC], bf16)
    ein("W1b", [108, 5 * 112], bf16)
    ein("W2b", [128, 5 * 104], bf16)
    ein("W3s", [120, 200], bf16)
    ein("Wouts", [80, 40], bf16)
    ein("A2T", [a1, 12], fp32)
    ein("A3T", [20, 30], fp32)
    ein("BW2", [12, 2], fp32)
    ein("BW3", [30, 3], fp32)
    ein("CW2", [2, 20], fp32)
    ein("CW3", [3, 20], fp32)
    ein("ONES2", [2, 1], fp32)
    ein("ONES3", [3, 1], fp32)
    ein("IND1", [16 * a1, a1], fp32)
    ein("IND2c", [40, 20], fp32)
    ein("EXP2", [20, 80], fp32)
    ein("EXPD3", [20, 120], fp32)
    ein("boutc", [10, 1], fp32)
    ein("zeros", [40, 16 * B_LOC], bf16)

    out_ext = nc.dram_tensor("out", [10, B_LOC], fp32, kind="ExternalOutput")

    with tile.TileContext(nc, num_cores=N_CORES) as tc:
        with (
            tc.tile_pool(name="const", bufs=1) as cpool,
            tc.tile_pool(name="work", bufs=1) as wpool,
            tc.tile_pool(name="psum", bufs=8, space="PSUM") as pp,
            tc.tile_pool(name="dram", bufs=1, space="DRAM") as dpool,
        ):
            # ------- X4 chunked load (sync queue) -------
            X4 = wpool.tile([108, 36 * B_LOC], bf16, tag="X4", name="X4")
            NCHUNK = 6
            xc = 36 // NCHUNK
            for ci in range(NCHUNK):
                c0 = ci * xc * B_LOC
                c1 = (ci + 1) * xc * B_LOC
                nc.sync.dma_start(X4[:, c0:c1], ext["X4"].ap()[:, c0:c1])

            # ------- constants (scalar queue; gpsimd reserved for collectives)
            def load(name, shape, dt, pool=cpool):
                t = pool.tile(shape, dt, tag=name, name=name)
                nc.scalar.dma_start(t[:], ext[name].ap())
                return t

            W1bb = load("W1b", [108, 5 * 112], bf16)
            W2bb = load("W2b", [128, 5 * 104], bf16)
            W3sb = load("W3s", [120, 200], bf16)
            Woutsb = load("Wouts", [80, 40], bf16)
            A2Tb = load("A2T", [a1, 12], fp32)
            A3Tb = load("A3T", [20, 30], fp32)
            BW2b = load("BW2", [12, 2], fp32)
            BW3b = load("BW3", [30, 3], fp32)
            CW2b = load("CW2", [2, 20], fp32)
            CW3b = load("CW3", [3, 20], fp32)
            ONES2b = load("ONES2", [2, 1], fp32)
            ONES3b = load("ONES3", [3, 1], fp32)
            IND1b = load("IND1", [16 * a1, a1], fp32)
            IND2cb = load("IND2c", [40, 20], fp32)
            EXP2b = load("EXP2", [20, 80], fp32)
            EXPD3b = load("EXPD3", [20, 120], fp32)
            boutb = load("boutc", [10, 1], fp32)
            one1 = cpool.tile([1, 1], fp32, tag="one1", name="one1")
            nc.vector.memset(one1[:], 1.0)

            # persistent tiles
            XPAB = wpool.tile([112, 16 * B_LOC], bf16, tag="XPAB", name="XPAB")
            XPB2 = wpool.tile([48, 16 * B_LOC], bf16, tag="XPB2", name="XPB2")
            P1u = wpool.tile([npar1, 16 * B_LOC], bf16, tag="P1u", name="P1u")
            P1 = wpool.tile([npar1, 16 * B_LOC], bf16, tag="P1", name="P1")
            S1acc = wpool.tile([npar1, 4], fp32, tag="S1acc", name="S1acc")
            S2 = wpool.tile([128, 20 * B_LOC], bf16, tag="S2", name="S2")
            XQAB = [wpool.tile([104, 8 * B_LOC], bf16, tag=f"XQAB_{w}",
                               name=f"XQAB_{w}") for w in range(4)]
            XQB2 = [wpool.tile([40, 8 * B_LOC], bf16, tag=f"XQB2_{w}",
                               name=f"XQB2_{w}") for w in range(4)]
            Q2u = [wpool.tile([40, 8 * B_LOC], bf16, tag=f"Q2u_{w}",
                              name=f"Q2u_{w}") for w in range(4)]
            S2acc = wpool.tile([40, 4], fp32, tag="S2acc", name="S2acc")
            S3 = [wpool.tile([120, 12 * B_LOC], bf16, tag=f"S3_{w}",
                             name=f"S3_{w}") for w in range(4)]
            XP3t = [wpool.tile([104, 4 * B_LOC], bf16, tag=f"XP3t_{t}",
                               name=f"XP3t_{t}") for t in range(2)]
            R3A = wpool.tile([80, 4 * B_LOC], bf16, tag="R3A", name="R3A")
            R3B = wpool.tile([80, 4 * B_LOC], bf16, tag="R3B", name="R3B")
            P3 = wpool.tile([80, 4 * B_LOC], bf16, tag="P3", name="P3")

            # x-halo zeroing only (y-halos handled by K-narrowing); vector is
            # idle this early
            nc.vector.memset(S2[:, 0:2 * B_LOC], 0.0)
            nc.vector.memset(S2[:, 18 * B_LOC:20 * B_LOC], 0.0)
            for w in range(4):
                nc.vector.memset(S3[w][:, 0:2 * B_LOC], 0.0)
                nc.vector.memset(S3[w][:, 10 * B_LOC:12 * B_LOC], 0.0)

            zerosb = ext["zeros"].ap()
            # y-halo rows: zero via DMA (cheap, off critical path)
            nc.sync.dma_start(S2[0:2 * a1, 2 * B_LOC:18 * B_LOC],
                              zerosb[0:2 * a1, :])
            nc.sync.dma_start(S2[96 + 6 * a1:96 + 8 * a1, 2 * B_LOC:18 * B_LOC],
                              zerosb[0:2 * a1, :])
            nc.scalar.dma_start(S3[0][0:40, 2 * B_LOC:10 * B_LOC],
                                zerosb[0:40, 0:8 * B_LOC])
            nc.scalar.dma_start(S3[3][80:120, 2 * B_LOC:10 * B_LOC],
                                zerosb[0:40, 0:8 * B_LOC])

            # PE warm-up burst (overlaps X4 DMA)
            for _ in range(2):
                wps = pp.tile([128, 320], fp32, tag="convps", name="convps")
                for ws in range(5):
                    nc.tensor.matmul(wps[0:112, :], X4[:, 0:112],
                                     X4[:, 0:320],
                                     start=(ws == 0), stop=(ws == 4))

            dps = pp.tile([128, 512], fp32, tag="convps", name="convps")

            def dummy(n=1):
                for _ in range(n):
                    nc.tensor.matmul(dps[0:108, :], W1bb[:, 0:108],
                                     W1bb[:, 0:512], start=True, stop=True)

            # ======================= LAYER 1 =======================
            # psum [112, 512]; lanes ypar*64 + yh*a1 + o; cols (x2, b256)
            for j in range(16):
                ps = pp.tile([128, 512], fp32, tag="convps", name="convps")
                for s in range(5):
                    nc.tensor.matmul(
                        ps[0:112, :],
                        W1bb[:, s * 112:(s + 1) * 112],
                        X4[:, (2 * j + s) * B_LOC:(2 * j + s + 2) * B_LOC],
                        start=(s == 0), stop=(s == 4))
                nc.vector.tensor_reduce(
                    out=XPAB[:, j * B_LOC:(j + 1) * B_LOC],
                    in_=ps[0:112, :].rearrange("p (x b) -> p b x", x=2),
                    axis=AX.X, op=AL.max)
                if j in (3, 7, 11, 15):
                    q = j // 4
                    qc = slice(q * 4 * B_LOC, (q + 1) * 4 * B_LOC)
                    eng = nc.sync if q % 2 == 0 else nc.scalar
                    eng.dma_start(XPB2[:, qc], XPAB[64:112, qc])
                    nc.vector.scalar_tensor_tensor(
                        out=P1[:, qc], in0=XPAB[0:npar1, qc], scalar=0.0,
                        in1=XPB2[0:npar1, qc], op0=AL.max, op1=AL.max,
                        accum_out=S1acc[:, q:q + 1])
                if j in (7, 15):
                    h = j // 8
                    hc = slice(h * 8 * B_LOC, (h + 1) * 8 * B_LOC)
                    # S2 staging for this x-half
                    for w in range(4):
                        ya, yb = max(0, 4 * w - 2), min(16, 4 * w + 6)
                        da = 32 * w + (ya - (4 * w - 2)) * a1
                        eng = nc.sync if w % 2 == 0 else nc.scalar
                        eng.dma_start(
                            S2[da:da + (yb - ya) * a1,
                               (2 + 8 * h) * B_LOC:(10 + 8 * h) * B_LOC],
                            P1[ya * a1:yb * a1, hc])

            # ---- s1 + AllReduce (emitted after L2 j2=1; PE never stalls) ----
            cc1_in = dpool.tile([a1, 1], fp32, tag="cc1_in", name="cc1_in")
            cc1_out = dpool.tile([a1, 1], fp32, tag="cc1_out", name="cc1_out")

            def s1_fold():
                s1ps = pp.tile([a1, 4], fp32, tag="convps", name="convps")
                nc.tensor.matmul(s1ps[:], IND1b[:], S1acc[:],
                                 start=True, stop=True)
                s1l2 = cpool.tile([a1, 4], fp32, tag="s1l2", name="s1l2")
                s1l = cpool.tile([a1, 1], fp32, tag="s1l", name="s1l")
                nc.scalar.activation(s1l2[:], s1ps[:], AF.Identity,
                                     accum_out=s1l[:])
                nc.gpsimd.dma_start(cc1_in[:], s1l[:])
                nc.gpsimd.collective_compute(
                    "AllReduce", AL.add,
                    replica_groups=[list(range(N_CORES))],
                    ins=[cc1_in.opt()], outs=[cc1_out.opt()])

            # ======================= LAYER 2 =======================
            # 4 w-slots row-packed; psum lanes par*64 + oyh*20 + oc
            # y-halo rows zeroed via zeros-DMA (32-align rule forbids
            # K-narrowed operand bases)
            def l2_j2(j2):
                pss2 = [pp.tile([128, 512], fp32, tag="convps",
                                name="convps") for _ in range(4)]
                for s in range(5):
                    for w in range(4):
                        nc.tensor.matmul(
                            pss2[w][0:104, :],
                            W2bb[32 * w:32 * w + nk2,
                                 s * 104:(s + 1) * 104],
                            S2[32 * w:32 * w + nk2,
                               (2 * j2 + s) * B_LOC:(2 * j2 + s + 2) * B_LOC],
                            start=(s == 0), stop=(s == 4),
                            tile_position=(32 * w, 0))
                for w in range(4):
                    nc.vector.tensor_reduce(
                        out=XQAB[w][:, j2 * B_LOC:(j2 + 1) * B_LOC],
                        in_=pss2[w][0:104, :].rearrange("p (x b) -> p b x", x=2),
                        axis=AX.X, op=AL.max)

            l2_j2(0)
            l2_j2(1)
            s1_fold()
            for j2 in range(2, 8):
                l2_j2(j2)

            # y-pool + relu + s2 accumulation (UNMASKED); relu split across
            # scalar (ACT, w odd) and vector (TS relu + reduce, w even)
            for w in range(4):
                eng = nc.sync if w % 2 == 0 else nc.scalar
                eng.dma_start(XQB2[w][:], XQAB[w][64:104, :])
                nc.vector.scalar_tensor_tensor(
                    out=Q2u[w][:], in0=XQAB[w][0:40, :], scalar=0.0,
                    in1=XQB2[w][:], op0=AL.max, op1=AL.max,
                    accum_out=S2acc[:, w:w + 1])

            # ---- s2 + AllReduce ----
            s2ps = pp.tile([20, 4], fp32, tag="convps", name="convps")
            nc.tensor.matmul(s2ps[:], IND2cb[:], S2acc[:], start=True, stop=True)
            s2loc = cpool.tile([20, 4], fp32, tag="s2loc", name="s2loc")
            s2l = cpool.tile([20, 1], fp32, tag="s2l", name="s2l")
            nc.scalar.activation(s2loc[:], s2ps[:], AF.Identity,
                                 accum_out=s2l[:])
            cc2_in = dpool.tile([20, 1], fp32, tag="cc2_in", name="cc2_in")
            cc2_out = dpool.tile([20, 1], fp32, tag="cc2_out", name="cc2_out")
            nc.sync.dma_start(cc2_in[:], s2l[:])
            nc.gpsimd.collective_compute(
                "AllReduce", AL.add, replica_groups=[list(range(N_CORES))],
                ins=[cc2_in.opt()], outs=[cc2_out.opt()])
            s1f = cpool.tile([a1, 1], fp32, tag="s1f", name="s1f")
            nc.gpsimd.dma_start(s1f[:], cc1_out[:])

            # ---- hash chain ----
            def hash_chain(v_sb, ATb, BWb, CWb, ONESb, LK, L, name,
                           warm=False):
                zps = pp.tile([LK, 1], fp32, tag="convps", name="convps")
                nc.tensor.matmul(zps[:], ATb[:], v_sb[:], start=True, stop=True)
                if warm:
                    dummy(2)
                bits = cpool.tile([LK, 1], fp32, tag=f"bits{name}", name=f"bits{name}")
                nc.vector.tensor_scalar(out=bits[:], in0=zps[:], scalar1=0.0,
                                        scalar2=None, op0=AL.is_gt)
                yps = pp.tile([L, 1], fp32, tag="convps", name="convps")
                nc.tensor.matmul(yps[:], BWb[:], bits[:], start=True, stop=True)
                if warm:
                    dummy(2)
                code = cpool.tile([L, 1], fp32, tag=f"code{name}", name=f"code{name}")
                nc.vector.tensor_copy(code[:], yps[:])
                ge = cpool.tile([L, 1], fp32, tag=f"ge{name}", name=f"ge{name}")
                for _ in range(2):
                    nc.vector.tensor_scalar(out=ge[:], in0=code[:],
                                            scalar1=24.5, scalar2=None,
                                            op0=AL.is_gt)
                    nc.vector.tensor_scalar(out=ge[:], in0=ge[:], scalar1=25.0,
                                            scalar2=None, op0=AL.mult)
                    nc.vector.tensor_tensor(out=code[:], in0=code[:],
                                            in1=ge[:], op=AL.subtract)
                eq = cpool.tile([L, 20], fp32, tag=f"eq{name}", name=f"eq{name}")
                nc.vector.tensor_tensor(out=eq[:], in0=CWb[:],
                                        in1=code[:].broadcast_to((L, 20)),
                                        op=AL.is_equal)
                cps = pp.tile([1, 20], fp32, tag="convps", name="convps")
                nc.tensor.matmul(cps[:], ONESb[:], eq[:], start=True, stop=True)
                mrow = cpool.tile([1, 20], fp32, tag=f"mrow{name}", name=f"mrow{name}")
                nc.vector.tensor_scalar(out=mrow[:], in0=cps[:], scalar1=0.5,
                                        scalar2=None, op0=AL.is_gt)
                mtp = pp.tile([20, 1], fp32, tag="convps", name="convps")
                nc.tensor.transpose(mtp[:], mrow[:], one1[:])
                mcol = cpool.tile([20, 1], fp32, tag=f"mcol{name}", name=f"mcol{name}")
                nc.vector.tensor_copy(mcol[:], mtp[:])
                return mcol

            # m2 from s1f; fold into W3 weights
            m2col = hash_chain(s1f, A2Tb, BW2b, CW2b, ONES2b, 12, 2, "m2",
                               warm=True)
            m2ps = pp.tile([120, 1], fp32, tag="convps", name="convps")
            nc.tensor.matmul(m2ps[:], EXPD3b[:], m2col[:], start=True, stop=True)
            dummy(4)
            m2r = cpool.tile([120, 1], fp32, tag="m2r", name="m2r")
            nc.vector.tensor_copy(m2r[:], m2ps[:])
            W3M = cpool.tile([120, 200], bf16, tag="W3M", name="W3M")
            nc.vector.tensor_scalar(out=W3M[:], in0=W3sb[:], scalar1=m2r[:],
                                    scalar2=None, op0=AL.mult)

            # ---- S3 staging (Q2u -> S3; no m2 dependency) ----
            _dq = 0
            for w3 in range(4):
                for w in range(max(0, w3 - 1), min(4, w3 + 2)):
                    da = (w - (w3 - 1)) * 40
                    eng = nc.sync if _dq % 2 == 0 else nc.scalar
                    _dq += 1
                    eng.dma_start(
                        S3[w3][da:da + 40, 2 * B_LOC:10 * B_LOC],
                        Q2u[w][:, :])
            s2f = cpool.tile([20, 1], fp32, tag="s2f", name="s2f")
            nc.gpsimd.dma_start(s2f[:], cc2_out[:])

            # ======================= LAYER 3 =======================

            def l3_t(t):
                pss3 = [pp.tile([128, 512], fp32, tag="convps",
                                name="convps") for _ in range(4)]
                for s in range(5):
                    for j3 in range(4):
                        for c in range(2):
                            nc.tensor.matmul(
                                pss3[j3][64 * c:64 * c + 40, :],
                                W3M[:, s * 40:(s + 1) * 40],
                                S3[2 * t + c][
                                    :, (2 * j3 + s) * B_LOC:
                                       (2 * j3 + s + 2) * B_LOC],
                                start=(s == 0), stop=(s == 4),
                                tile_position=(0, 64 * c))
                for j3 in range(4):
                    nc.vector.tensor_reduce(
                        out=XP3t[t][:, j3 * B_LOC:(j3 + 1) * B_LOC],
                        in_=pss3[j3][0:104, :].rearrange(
                            "p (x b) -> p b x", x=2),
                        axis=AX.X, op=AL.max)
                # per-t y-pool parity DMAs (overlap next t's matmuls)
                _dq3 = 0
                for c in range(2):
                    w3 = 2 * t + c
                    for par, dstt in ((0, R3A), (1, R3B)):
                        eng = nc.sync if _dq3 % 2 == 0 else nc.scalar
                        _dq3 += 1
                        eng.dma_start(
                            dstt[20 * w3:20 * w3 + 20, :],
                            XP3t[t][64 * c + 20 * par:64 * c + 20 * par + 20,
                                    :])

            l3_t(0)
            l3_t(1)

            # fused y-pool + relu -> P3
            nc.vector.scalar_tensor_tensor(
                out=P3[:], in0=R3A[:], scalar=0.0, in1=R3B[:],
                op0=AL.max, op1=AL.max)

            # ---- m3 chain ----
            v3 = cpool.tile([20, 1], fp32, tag="v3", name="v3")
            nc.vector.tensor_tensor(out=v3[:], in0=s2f[:], in1=m2col[:],
                                    op=AL.mult)
            m3col = hash_chain(v3, A3Tb, BW3b, CW3b, ONES3b, 30, 3, "m3")
            m3ps = pp.tile([80, 1], fp32, tag="convps", name="convps")
            nc.tensor.matmul(m3ps[:], EXP2b[:], m3col[:], start=True, stop=True)
            m3exp = cpool.tile([80, 1], fp32, tag="m3exp", name="m3exp")
            nc.vector.tensor_copy(m3exp[:], m3ps[:])
            WoutM = cpool.tile([80, 40], bf16, tag="WoutM", name="WoutM")
            nc.vector.tensor_scalar(out=WoutM[:], in0=Woutsb[:],
                                    scalar1=m3exp[:], scalar2=None,
                                    op0=AL.mult)
            fcps = pp.tile([10, 256], fp32, tag="convps", name="convps")
            for xq in range(4):
                nc.tensor.matmul(fcps[:],
                                 WoutM[:, xq * 10:(xq + 1) * 10],
                                 P3[:, xq * B_LOC:(xq + 1) * B_LOC],
                                 start=(xq == 0), stop=(xq == 3))
            out_sb = cpool.tile([10, 256], fp32, tag="out_sb", name="out_sb")
            nc.scalar.activation(out_sb[:], fcps[:], AF.Identity, bias=boutb[:])
            nc.sync.dma_start(out_ext.ap(), out_sb[:])

    nc.compile()
    return nc


# ======== baseline fallback (a1 > 4) ========
def _host_prep_slow(x, W1, A1, W2, A2, W3, A3, Wout, bout, force_masks=False):
    """Build all per-core / shared device tensors. Returns (shared, per_core list)."""
    x = np.asarray(x, np.float32)
    W1 = np.asarray(W1, np.float32)
    W2 = np.asarray(W2, np.float32)
    W3 = np.asarray(W3, np.float32)
    Wout = np.asarray(Wout, np.float32)
    bout = np.asarray(bout, np.float32)

    # ---- layer-1 mask from full-batch channel means (host) ----
    m1 = _query_mask(x.mean((0, 2, 3)), W1, A1).astype(np.float32)  # (16,)
    if force_masks:
        m1 = np.ones(16, np.float32)

    # ---- X staging: [128, 2*256*36] bf16 ----
    # quadrant r rows (iy8, ic3) iy-major; windows w = c*4 + r at wsel=c
    xs_all = []
    xsh = x.reshape(N_CORES, B_LOC, 3, 32, 32)
    xpad = np.zeros((N_CORES, B_LOC, 3, 36, 36), np.float32)
    xpad[:, :, :, 2:34, 2:34] = xsh
    for core in range(N_CORES):
        X4 = np.zeros((128, 2 * B_LOC * 36), np.float32)
        for w in range(8):
            r, c = w % 4, w // 4
            for iy in range(8):
                gy = 4 * w - 2 + iy + 2  # row in padded (0..35)
                for ic in range(3):
                    row = 32 * r + iy * 3 + ic
                    X4[row, c * (B_LOC * 36):(c + 1) * (B_LOC * 36)] = \
                        xpad[core, :, ic, gy, :].reshape(-1)
        xs_all.append(X4.astype(BF))

    # ---- weight stagings ----
    # L1: W1s [128, 5*64]: quadrant r rows (iy8,ic3); col (s*64 + par*32+oyh*16+oc)
    W1s = np.zeros((128, 5 * 64), np.float32)
    for r in range(4):
        for iy in range(8):
            for ic in range(3):
                row = 32 * r + iy * 3 + ic
                for s in range(5):
                    for par in range(2):
                        for oyh in range(2):
                            oy = 2 * oyh + par
                            ky = iy - oy
                            if 0 <= ky < 5:
                                for oc in range(16):
                                    W1s[row, s * 64 + par * 32 + oyh * 16 + oc] = \
                                        W1[oc, ic, ky, s]
    # L2: W2s [128, 5*80]: rows (iy8, ic16) iy-major; lane par*40+oyh*20+oc
    W2s = np.zeros((128, 5 * 80), np.float32)
    for iy in range(8):
        for ic in range(16):
            row = iy * 16 + ic
            for s in range(5):
                for par in range(2):
                    for oyh in range(2):
                        oy = 2 * oyh + par
                        ky = iy - oy
                        if 0 <= ky < 5:
                            for oc in range(20):
                                W2s[row, s * 80 + par * 40 + oyh * 20 + oc] = \
                                    W2[oc, ic, ky, s]
    # L3: W3s [120, 5*40]: rows (iy6, ic20) iy-major; lane par*20+oc
    W3s = np.zeros((120, 5 * 40), np.float32)
    for iy in range(6):
        for ic in range(20):
            row = iy * 20 + ic
            for s in range(5):
                for par in range(2):
                    ky = iy - par
                    if 0 <= ky < 5:
                        for oc in range(20):
                            W3s[row, s * 40 + par * 20 + oc] = W3[oc, ic, ky, s]
    # FC: Wouts [80, 4*10]: rows (y'4, oc20) = y'*20+oc; col x'*10+co
    Wouts = np.zeros((80, 40), np.float32)
    for oc in range(20):
        for yq in range(4):
            for xq in range(4):
                for co in range(10):
                    Wouts[yq * 20 + oc, xq * 10 + co] = Wout[co, oc * 16 + yq * 4 + xq]

    # ---- hash constants ----
    At2 = _collapse_A(A2, 16) * m1[None, None, :]       # (2,6,16), m1 folded
    A2T = At2.transpose(2, 0, 1).reshape(16, 12).copy() # rows c, cols (l,k)
    At3 = _collapse_A(A3, 20)                           # (3,10,20)
    A3T = At3.transpose(2, 0, 1).reshape(20, 30).copy()
    BW2 = np.zeros((12, 2), np.float32)
    for l in range(2):
        for k in range(6):
            BW2[l * 6 + k, l] = float((2 ** k) % TABLE)
    BW3 = np.zeros((30, 3), np.float32)
    for l in range(3):
        for k in range(10):
            BW3[l * 10 + k, l] = float((2 ** k) % TABLE)
    CW2 = _filter_codes(W2, A2).astype(np.float32)      # (2,20)
    CW3 = _filter_codes(W3, A3).astype(np.float32)      # (3,20)
    ONES2 = np.ones((2, 1), np.float32)
    ONES3 = np.ones((3, 1), np.float32)
    # P1 lanes: r4*32 + oyh2*16 + oc16 -> oc = lane % 16
    IND1 = np.zeros((128, 16), np.float32)
    for lane in range(128):
        IND1[lane, lane % 16] = 1.0
    # Q2 lanes: q2*40 + oyh2*20 + oc20 -> oc = lane % 20
    IND2 = np.zeros((80, 20), np.float32)
    for lane in range(80):
        IND2[lane, lane % 20] = 1.0
    # EXP2 [20,80]: m[20] -> 80 lanes with oc = lane % 20 (Q2 and P3 layouts)
    EXP2 = np.zeros((20, 80), np.float32)
    for lane in range(80):
        EXP2[lane % 20, lane] = 1.0
    # m1 expanded to P1 lanes (oc = lane % 16)
    m1exp = np.tile(m1, 8)[:, None].astype(np.float32)  # (128,1)
    # mean scales folded: reference cm2 = sum(h1)/(2048*16*16); A2T already has m1.
    # sign-invariant to positive scale -> no normalization needed.
    boutc = bout.reshape(10, 1).astype(np.float32)

    if force_masks:
        BW2[:] = 0.0; BW3[:] = 0.0; CW2 = np.zeros_like(CW2); CW3 = np.zeros_like(CW3)
    shared = dict(
        W1s=W1s.astype(BF), W2s=W2s.astype(BF), W3s=W3s.astype(BF),
        Wouts=Wouts.astype(BF),
        A2T=A2T, A3T=A3T, BW2=BW2, BW3=BW3, CW2=CW2, CW3=CW3,
        ONES2=ONES2, ONES3=ONES3, IND1=IND1, IND2=IND2, EXP2=EXP2,
        m1exp=m1exp, boutc=boutc,
        zeros=np.zeros((128, 5120), BF),
    )
    return shared, xs_all



def build_nc_slow(debug=False):
    nc = bacc.Bacc("TRN2", target_bir_lowering=False, debug=False,
                   num_devices=N_CORES)

    ext = {}
    def ein(name, shape, dt):
        ext[name] = nc.dram_tensor(name, shape, dt, kind="ExternalInput")
        return ext[name]

    ein("X4", [128, 2 * B_LOC * 36], bf16)
    ein("W1s", [128, 320], bf16)
    ein("W2s", [128, 400], bf16)
    ein("W3s", [120, 200], bf16)
    ein("Wouts", [80, 40], bf16)
    ein("A2T", [16, 12], fp32)
    ein("A3T", [20, 30], fp32)
    ein("BW2", [12, 2], fp32)
    ein("BW3", [30, 3], fp32)
    ein("CW2", [2, 20], fp32)
    ein("CW3", [3, 20], fp32)
    ein("ONES2", [2, 1], fp32)
    ein("ONES3", [3, 1], fp32)
    ein("IND1", [128, 16], fp32)
    ein("IND2", [80, 20], fp32)
    ein("EXP2", [20, 80], fp32)
    ein("m1exp", [128, 1], fp32)
    ein("boutc", [10, 1], fp32)
    ein("zeros", [128, 5120], bf16)

    out_ext = nc.dram_tensor("out", [10, B_LOC], fp32, kind="ExternalOutput")
    dbg = {}
    if debug:
        for nm, shape in [("d_XP1_0", [128, 4096]), ("d_P1_0", [128, 4096]),
                          ("d_P1_1", [128, 4096]),
                           ("d_XP2_0", [128, 2048]),
                          ("d_Q2_0", [80, 2048]), ("d_Q2_1", [80, 2048]),
                          ("d_S3_1", [120, 3072]),
                          ("d_XP3_0", [128, 1024]), ("d_P3", [80, 1024]),
                          ("d_m", [80, 4])]:
            dbg[nm] = nc.dram_tensor(nm, shape, fp32, kind="ExternalOutput")

    with tile.TileContext(nc, num_cores=N_CORES) as tc:
        with (
            tc.tile_pool(name="const", bufs=1) as cpool,
            tc.tile_pool(name="work", bufs=1) as wpool,
            tc.tile_pool(name="scratch", bufs=(3 if debug else 6)) as spool,
            tc.tile_pool(name="dbgpool", bufs=1) as dbgpool,
            tc.tile_pool(name="psum", bufs=8, space="PSUM") as pp,
            tc.tile_pool(name="dram", bufs=1, space="DRAM") as dpool,
        ):
            # ------- X4 load first (largest, gates L1) -------
            X4g = wpool.tile([128, 2 * B_LOC * 36], bf16, tag="X4",
                             name="X4")
            nc.sync.dma_start(X4g[:], ext["X4"].ap())

            # ------- load constants (gpsimd queue; sync stays free) -------
            def load(name, shape, dt, pool=cpool):
                t = pool.tile(shape, dt, tag=name)
                nc.gpsimd.dma_start(t[:], ext[name].ap())
                return t

            W1sb = load("W1s", [128, 320], bf16)
            W2sb = load("W2s", [128, 400], bf16)
            W3sb = load("W3s", [120, 200], bf16)
            Woutsb = load("Wouts", [80, 40], bf16)
            A2Tb = load("A2T", [16, 12], fp32)
            A3Tb = load("A3T", [20, 30], fp32)
            BW2b = load("BW2", [12, 2], fp32)
            BW3b = load("BW3", [30, 3], fp32)
            CW2b = load("CW2", [2, 20], fp32)
            CW3b = load("CW3", [3, 20], fp32)
            ONES2b = load("ONES2", [2, 1], fp32)
            ONES3b = load("ONES3", [3, 1], fp32)
            IND1b = load("IND1", [128, 16], fp32)
            IND2b = load("IND2", [80, 20], fp32)
            EXP2b = load("EXP2", [20, 80], fp32)
            m1expb = load("m1exp", [128, 1], fp32)
            boutb = load("boutc", [10, 1], fp32)
            one1 = cpool.tile([1, 1], fp32, tag="one1", name="one1")
            nc.vector.memset(one1[:], 1.0)

            # PE warm-up burst (overlaps the X4 DMA; warms the HAM clock)
            for _ in range(3):
                wps = pp.tile([128, 320], fp32, tag="convps", name="convps")
                for ws in range(5):
                    nc.tensor.matmul(wps[:], W1sb[:, 0:128], W1sb[:],
                                     start=(ws == 0), stop=(ws == 4))

            zext = ext["zeros"].ap()

            def dump(nm, t):
                if not debug:
                    return
                f = dbgpool.tile(list(t.shape), bf16, tag="dumpf", name="dumpf")
                nc.vector.tensor_copy(f[:], t[:])
                nc.gpsimd.dma_start(dbg[nm].ap(), f[:])

            # persistent pooled-activation tiles (outer pool)
            P1 = [wpool.tile([128, 16 * 256], bf16, tag=f"P1_{c}", name=f"P1_{c}")
                  for c in range(2)]
            S1acc = wpool.tile([128, 2], fp32, tag="S1acc", name="S1acc")
            Q2 = [wpool.tile([80, 8 * 256], bf16, tag=f"Q2_{h}", name=f"Q2_{h}") for h in range(2)]
            S2 = [wpool.tile([128, 20 * B_LOC], bf16, tag=f"S2_{w}",
                             name="S2") for w in range(4)]
            S3 = [wpool.tile([120, 12 * B_LOC], bf16, tag=f"S3_{w}",
                             name="S3") for w in range(4)]
            # pad DMAs issued up-front (depend on nothing)
            for w in range(4):
                dpad = S2[w].rearrange("p (blk q) -> p blk q", q=512)
                nc.sync.dma_start(dpad[:, 0:10:9, :],
                                  zext[0:128, 0:1024].rearrange(
                                      "p (a q) -> p a q", q=512))
                dpad3 = S3[w].rearrange("p (blk q) -> p blk q", q=512)
                nc.sync.dma_start(dpad3[:, 0:6:5, :],
                                  zext[0:120, 0:1024].rearrange(
                                      "p (a q) -> p a q", q=512))
            nc.sync.dma_start(S2[0][0:32, 2 * B_LOC:18 * B_LOC],
                              zext[0:32, 0:16 * B_LOC])
            nc.sync.dma_start(S2[3][96:128, 2 * B_LOC:18 * B_LOC],
                              zext[0:32, 0:16 * B_LOC])
            nc.sync.dma_start(S3[0][0:40, 2 * B_LOC:10 * B_LOC],
                              zext[0:40, 0:8 * B_LOC])
            nc.sync.dma_start(S3[3][80:120, 2 * B_LOC:10 * B_LOC],
                              zext[0:40, 0:8 * B_LOC])
            S2acc = wpool.tile([80, 2], fp32, tag="S2acc", name="S2acc")
            P3 = wpool.tile([80, 4 * 256], bf16, tag="P3", name="P3")

            # ======================= LAYER 1 =======================
            with tc.tile_pool(name="l1big", bufs=1) as l1pool:
                X4 = X4g
                XP1 = [l1pool.tile([128, 16 * 256], bf16, tag=f"XP1_{r}", name=f"XP1_{r}")
                       for r in range(4)]

                tile_count = 0
                v = X4.rearrange("p (w b x) -> p w b x", w=2, b=B_LOC)
                for j in range(16):
                    pss = [pp.tile([128, 512], fp32, tag="convps",
                                   name="convps") for _ in range(4)]
                    for s in range(5):
                        for c in range(2):
                            for r in range(4):
                                nc.tensor.matmul(
                                    pss[r][64 * c:64 * c + 64, :],
                                    W1sb[32 * r:32 * r + 24,
                                         s * 64:(s + 1) * 64],
                                    v[32 * r:32 * r + 24, c, :,
                                      2 * j + s:2 * j + s + 2],
                                    start=(s == 0), stop=(s == 4),
                                    tile_position=(32 * r, 64 * c))
                    for r in range(4):
                        ps = pss[r]
                        use_act = (tile_count % 20) < int(ACT_EVAC_FRAC * 20)
                        tile_count += 1
                        if use_act:
                            sc = spool.tile([128, 512], bf16, tag="evac",
                                            name="evac")
                            nc.scalar.activation(sc[:], ps[:], AF.Copy)
                            vv = sc.rearrange("p (b x) -> p b x", x=2)
                            nc.vector.tensor_tensor(
                                out=XP1[r][:, j * 256:j * 256 + 256],
                                in0=vv[:, :, 0], in1=vv[:, :, 1], op=AL.max)
                        else:
                            nc.vector.tensor_reduce(
                                out=XP1[r][:, j * 256:j * 256 + 256],
                                in_=ps.rearrange("p (b x) -> p b x", x=2),
                                axis=AX.X, op=AL.max)

                # y-pool: parity-split DMAs then TT max -> P1
                P1A = [l1pool.tile([128, 16 * 256], bf16, tag=f"P1A_{c}", name=f"P1A_{c}")
                       for c in range(2)]
                P1B = [l1pool.tile([128, 16 * 256], bf16, tag=f"P1B_{c}", name=f"P1B_{c}")
                       for c in range(2)]
                # P1 lanes: 32r + 16oyh + oc (y' = 8c + 2r + oyh)
                _dq = 0
                for r in range(4):
                    for c in range(2):
                        for par, dstt in ((0, P1A), (1, P1B)):
                            eng = nc.sync if _dq % 2 == 0 else nc.gpsimd
                            _dq += 1
                            eng.dma_start(
                                dstt[c][32 * r:32 * r + 32, :],
                                XP1[r][64 * c + 32 * par:
                                       64 * c + 32 * par + 32, :])
                dump("d_XP1_0", XP1[0])
                for c in range(2):
                    nc.vector.tensor_tensor(out=P1[c][:], in0=P1A[c][:],
                                            in1=P1B[c][:], op=AL.max)
                    nc.scalar.activation(P1[c][:], P1[c][:], AF.Relu,
                                         scale=m1expb[:],
                                         accum_out=S1acc[:, c:c + 1])
                dump("d_P1_0", P1[0])
                dump("d_P1_1", P1[1])

            # ---- s1 partial + AllReduce ----
            s1ps = pp.tile([16, 2], fp32, tag="convps", name="convps")
            nc.tensor.matmul(s1ps[:], IND1b[:], S1acc[:], start=True, stop=True)
            s1loc = cpool.tile([16, 2], fp32, tag="s1loc", name="s1loc")
            nc.vector.tensor_copy(s1loc[:], s1ps[:])
            s1l = cpool.tile([16, 1], fp32, tag="s1l", name="s1l")
            nc.vector.tensor_tensor(out=s1l[:], in0=s1loc[:, 0:1],
                                    in1=s1loc[:, 1:2], op=AL.add)
            cc1_in = dpool.tile([16, 1], fp32)
            cc1_out = dpool.tile([16, 1], fp32)
            nc.sync.dma_start(cc1_in[:], s1l[:])
            nc.gpsimd.collective_compute(
                "AllReduce", AL.add, replica_groups=[list(range(N_CORES))],
                ins=[cc1_in.opt()], outs=[cc1_out.opt()])
            s1f = cpool.tile([16, 1], fp32, tag="s1f", name="s1f")
            nc.sync.dma_start(s1f[:], cc1_out[:])

            # ---- hash chain ----
            def hash_chain(v_sb, ATb, BWb, CWb, ONESb, LK, L, name):
                zps = pp.tile([LK, 1], fp32, tag="convps", name="convps")
                nc.tensor.matmul(zps[:], ATb[:], v_sb[:], start=True, stop=True)
                bits = cpool.tile([LK, 1], fp32, tag=f"bits{name}", name=f"bits{name}")
                nc.vector.tensor_scalar(out=bits[:], in0=zps[:], scalar1=0.0,
                                        scalar2=None, op0=AL.is_gt)
                yps = pp.tile([L, 1], fp32, tag="convps", name="convps")
                nc.tensor.matmul(yps[:], BWb[:], bits[:], start=True, stop=True)
                code = cpool.tile([L, 1], fp32, tag=f"code{name}", name=f"code{name}")
                nc.vector.tensor_copy(code[:], yps[:])
                ge = cpool.tile([L, 1], fp32, tag=f"ge{name}", name=f"ge{name}")
                for _ in range(2):
                    nc.vector.tensor_scalar(out=ge[:], in0=code[:],
                                            scalar1=24.5, scalar2=None,
                                            op0=AL.is_gt)
                    nc.vector.tensor_scalar(out=ge[:], in0=ge[:], scalar1=25.0,
                                            scalar2=None, op0=AL.mult)
                    nc.vector.tensor_tensor(out=code[:], in0=code[:],
                                            in1=ge[:], op=AL.subtract)
                eq = cpool.tile([L, 20], fp32, tag=f"eq{name}", name=f"eq{name}")
                nc.vector.tensor_tensor(out=eq[:], in0=CWb[:],
                                        in1=code[:].broadcast_to((L, 20)),
                                        op=AL.is_equal)
                cps = pp.tile([1, 20], fp32, tag="convps", name="convps")
                nc.tensor.matmul(cps[:], ONESb[:], eq[:], start=True, stop=True)
                mrow = cpool.tile([1, 20], fp32, tag=f"mrow{name}", name=f"mrow{name}")
                nc.vector.tensor_scalar(out=mrow[:], in0=cps[:], scalar1=0.5,
                                        scalar2=None, op0=AL.is_gt)
                mtp = pp.tile([20, 1], fp32, tag="convps", name="convps")
                nc.tensor.transpose(mtp[:], mrow[:], one1[:])
                mcol = cpool.tile([20, 1], fp32, tag=f"mcol{name}", name=f"mcol{name}")
                nc.vector.tensor_copy(mcol[:], mtp[:])
                return mcol

            # ======================= LAYER 2 =======================
            with tc.tile_pool(name="l2big", bufs=1) as l2pool:
                # S2_w [128=(iy8,ic16), (xp20, b256)]
                # S2 valid-row staging (pads already zeroed up-front)
                for w in range(4):
                    y0, y1 = max(0, 4 * w - 2), min(16, 4 * w + 6)
                    iy0 = y0 - (4 * w - 2)
                    for ch in range(2):
                        ya, yb = max(y0, 8 * ch), min(y1, 8 * ch + 8)
                        if ya >= yb:
                            continue
                        la = 16 * (ya % 8)
                        da = 16 * (iy0 + (ya - y0))
                        nc.sync.dma_start(
                            S2[w][da:da + 16 * (yb - ya),
                                  2 * B_LOC:18 * B_LOC],
                            P1[ch][la:la + 16 * (yb - ya), :])

                XP2 = [l2pool.tile([128, 8 * 256], bf16, tag=f"XP2_{w}", name=f"XP2_{w}")
                       for w in range(4)]
                tile_count = 0
                for w in range(4):
                    rhsv = S2[w].rearrange("p (xp b) -> p b xp", b=B_LOC)
                    for jg in range(2):
                        pss = [pp.tile([128, 512], fp32, tag="convps",
                                       name="convps") for _ in range(4)]
                        for srt in range(5):
                            for jj in range(4):
                                j = 4 * jg + jj
                                nc.tensor.matmul(
                                    pss[jj][0:80, :],
                                    W2sb[:, srt * 80:(srt + 1) * 80],
                                    rhsv[:, :, 2 * j + srt:2 * j + srt + 2],
                                    start=(srt == 0), stop=(srt == 4))
                        for jj in range(4):
                            j = 4 * jg + jj
                            ps = pss[jj]
                            use_act = (tile_count % 20) < int(ACT_EVAC_FRAC * 20)
                            tile_count += 1
                            if use_act:
                                sc = spool.tile([128, 512], bf16, tag="evac",
                                                name="evac")
                                nc.scalar.activation(sc[0:80, :], ps[0:80, :],
                                                     AF.Copy)
                                vv = sc.rearrange("p (b x) -> p b x", x=2)
                                nc.vector.tensor_tensor(
                                    out=XP2[w][0:80, j * 256:j * 256 + 256],
                                    in0=vv[0:80, :, 0], in1=vv[0:80, :, 1],
                                    op=AL.max)
                            else:
                                nc.vector.tensor_reduce(
                                    out=XP2[w][0:80, j * 256:j * 256 + 256],
                                    in_=ps[0:80, :].rearrange(
                                        "p (b x) -> p b x", x=2),
                                    axis=AX.X, op=AL.max)

                m2col = hash_chain(s1f, A2Tb, BW2b, CW2b, ONES2b,
                                   12, 2, "m2")
                m2ps = pp.tile([80, 1], fp32, tag="convps", name="convps")
                nc.tensor.matmul(m2ps[:], EXP2b[:], m2col[:],
                                 start=True, stop=True)
                m2exp = cpool.tile([80, 1], fp32, tag="m2exp", name="m2exp")
                nc.vector.tensor_copy(m2exp[:], m2ps[:])

                Q2A = [l2pool.tile([80, 8 * 256], bf16, tag=f"Q2A_{h}",
                                   name="Q2A") for h in range(2)]
                Q2B = [l2pool.tile([80, 8 * 256], bf16, tag=f"Q2B_{h}",
                                   name="Q2B") for h in range(2)]
                # Q2 lanes: 40q + 20oyh + oc (y' = 4h + 2q + oyh, q = w % 2)
                _dq = 0
                for w in range(4):
